# revision 19
# baseline (speedup 1.0000x reference)
"""Two-layer GAT (PyG GATConv semantics) on 8 TRN2 NeuronCores.

Strategy (edge/graph parallel, v3):
  - Host (index manipulation only): sort non-self-loop edges by dst, shard
    dst nodes contiguously across the 8 cores, pad each dst tile's edge
    list to a multiple of 128, and precompute each 128-edge tile's 0/1
    incidence matrices S (dst x edge) and S^T as bf16 (pure index data).
    The host also pre-permutes x rows into per-edge-tile transposed blocks
    XET[c] = x[src[tile c]].T (a gather/reshape of the input - no
    arithmetic) so layer 1 needs NO device-side indirect DMA.  sst/XET are
    stored chunk-blocked so each load is ~128 descriptors of 1-2KB.
  - Phase A (per core, own dst shard): HAUGs[v] = [x@W1 | a_src | a_dst]
    (272 cols, bf16); bias b1 is applied post-softmax in the flush.
  - Phase B (edge phase, layer 1): per 128-edge tile, he = XET^T @ W1aug
    on the tensor engine; alpha_dst per edge via a small matmul
    S^T @ a_dst_tile; exp(leakyrelu(asrc+adst)); one incidence matmul
    accumulates [sum exp*h | sum exp] in PSUM.  Self-loops are added
    densely in the flush.  Normalize, +b1, ELU, store z; layer-2 rows
    T2[v] = [z@W2 + b2 | a2_src | a2_dst] computed inline (Phase C).
  - The T2 AllGather is split in two halves; layer-2 edges are bucketed by
    src half.  Bucket-0 T2 rows are indirect-gathered and staged to DRAM
    concurrently with the second half of Phase B (GpSimd is otherwise
    idle there), so only bucket-1 gathers remain serialized in Phase D.
  - Phase D: layer-2 edge phase (heads=1) -> output shard.

All floating-point math happens on device; the host only reorders
indices/rows and pads/reshapes layouts.
"""

import contextlib

import numpy as np

import concourse.bass as bass
import concourse.bacc as bacc
import concourse.mybir as mybir
import concourse.tile as tile
from concourse.bass_utils import run_bass_kernel_spmd

# ---- fixed problem hyperparameters (from the nn.Module) ----
F_IN = 256
H = 8
C = 32
NCLS = 40
NEG = 0.2

W = 8               # cores
P = 128             # partitions
D1 = F_IN + 2 * H   # HAUG row: [h (256) | a_src (8) | a_dst (8)] = 272
D2 = NCLS + 8       # T2 row: [h2+b2 (40) | a2_src (1) | a2_dst x7 (41:48)]
R = 4               # edge tiles batched per compute group

f32 = mybir.dt.float32
bf16 = mybir.dt.bfloat16
i32 = mybir.dt.int32
BF_NP = mybir.dt.np(bf16)

Exp = mybir.ActivationFunctionType.Exp
Copy = mybir.ActivationFunctionType.Copy
ADD = mybir.AluOpType.add
MULT = mybir.AluOpType.mult
MAX = mybir.AluOpType.max

TRACE = False       # set by test harness for profiling runs
_CACHE = {}


def _host_prep(x, edge_index):
    """Index-only preprocessing. Returns (meta, per-core arrays)."""
    N = x.shape[0]
    E = edge_index.shape[1]
    src_a = np.asarray(edge_index[0], np.int64)
    dst_a = np.asarray(edge_index[1], np.int64)
    order = np.argsort(dst_a, kind="stable")
    src_s = src_a[order].astype(np.int64)
    dst_s = dst_a[order].astype(np.int64)

    nt_real = -(-N // P)
    T = -(-nt_real // W)
    SH = T * P
    SH2 = SH // 2
    NPAD = W * SH

    # bucket edges by (core, dst tile, src half) for the split AllGather
    gt = dst_s // P                      # global dst tile
    half = (src_s % SH) // SH2           # 0 / 1
    key = gt * 2 + half
    cnt = np.bincount(key, minlength=W * T * 2).reshape(W, T, 2)
    K01 = -(-cnt.max(axis=0) // P)       # [T, 2] tiles per bucket
    K01[:, 0] = np.maximum(K01[:, 0], 1)
    K0 = K01[:, 0]
    K1 = K01[:, 1]
    K = K0 + K1
    offs = np.zeros(T + 1, np.int64)
    offs[1:] = np.cumsum(K)
    CT = int(offs[-1])

    # rank of each edge within its (core, tile, half) bucket
    sort2 = np.argsort(key, kind="stable")
    src_s = src_s[sort2]
    dst_s = dst_s[sort2]
    gt = gt[sort2]
    half = half[sort2]
    key = key[sort2]
    start_k = np.zeros(W * T * 2 + 1, np.int64)
    start_k[1:] = np.cumsum(cnt.reshape(-1))
    q = np.arange(len(dst_s)) - start_k[key]
    c_all = gt // T
    t_all = gt % T
    col = offs[t_all] + np.where(half == 0, 0, K0[t_all]) + q // P
    p_all = q % P

    src_arr = np.zeros((W, P, CT), np.int64)
    dstl_arr = np.full((W, P, CT), -1, np.int16)
    src_arr[c_all, p_all, col] = src_s
    dstl_arr[c_all, p_all, col] = (dst_s % P).astype(np.int16)

    # layer-2 gather rows in the split-T2F layout:
    # node v = s*SH + l -> T2F[l // SH2] row s*SH2 + (l % SH2)
    t2row = (src_arr // SH) * SH2 + (src_arr % SH) % SH2
    t2row = t2row.astype(np.int32)
    src_arr = src_arr.astype(np.int32)

    # chunk table: one entry per (t, j0) compute group
    chunks = []                          # (t, base, j0, r)
    chunk_of = {}
    for t in range(T):
        for j0 in range(0, int(K[t]), R):
            r = min(R, int(K[t]) - j0)
            chunk_of[(t, j0)] = len(chunks)
            chunks.append((t, int(offs[t]), j0, r))
    NCHT = len(chunks)

    offs0 = np.zeros(T + 1, np.int64)
    offs0[1:] = np.cumsum(K0)
    CT0 = int(offs0[-1])

    CMAX = max(-(-int(k) // R) for k in K)
    meta = dict(N=N, E=E, T=T, SH=SH, SH2=SH2, NPAD=NPAD,
                K=tuple(int(k) for k in K), K0=tuple(int(k) for k in K0),
                offs=offs, offs0=offs0, CT=CT, CT0=CT0, NCHT=NCHT,
                chunks=tuple(chunks), chunk_of=chunk_of, CMAX=CMAX)

    xb = np.zeros((NPAD, F_IN), BF_NP)
    xb[:N] = np.asarray(x, np.float32).astype(BF_NP)

    def build_xetc(c):
        # [NCHT, P(feat-half), 2, R, P(edge)] chunk-blocked transposed x rows
        out = np.zeros((NCHT, P, 2, R, P), BF_NP)
        for ci, (t, base, j0, r) in enumerate(chunks):
            cols = src_arr[c][:, base + j0:base + j0 + r]     # [P(edge), r]
            blk = xb[cols]                                    # [Pe, r, 256]
            out[ci, :, :, :r, :] = (blk.transpose(2, 1, 0)    # [256f, r, Pe]
                                    .reshape(2, P, r, P)      # [k, f, r, e]
                                    .transpose(1, 0, 2, 3))   # [f, k, r, e]
        return out

    xts = [np.ascontiguousarray(xb[c * SH:(c + 1) * SH].T) for c in range(W)]

    arrays = dict(src_arr=src_arr, dstl_arr=dstl_arr, t2row=t2row,
                  build_xetc=build_xetc, xts=xts)
    return meta, arrays


def _make_sstc(dstl_core, meta):
    """[P, CT] int16 dst-local (-1 pad) -> [NCHT, P, R, 256] bf16 [S | S^T]."""
    chunks = meta["chunks"]
    NCHT = meta["NCHT"]
    iota = np.arange(P, dtype=np.int16)
    out = np.zeros((NCHT, P, R, 2 * P), BF_NP)
    for ci, (t, base, j0, r) in enumerate(chunks):
        d = dstl_core[:, base + j0:base + j0 + r].T          # [r, 128e]
        S = (d[:, None, :] == iota[None, :, None])           # [r, d, e]
        out[ci, :, :r, :P] = S.transpose(1, 0, 2)
        out[ci, :, :r, P:] = S.transpose(2, 0, 1)
    return out


def _edge_chunks(nc, meta, consts, *, t, selfrow, getg, dglen, nheads, hw,
                 agg):
    """Shared per-chunk compute: attention weights + incidence aggregation.

    The whole tile group's incidence matrices load in ONE DMA (HWDGE issue
    cost is ~fixed per dma_start, so fewer+bigger transfers win).
    """
    K = meta["K"]
    chunk_of = meta["chunk_of"]
    CMAX = meta["CMAX"]
    sstc_d = consts["sstc_d"]
    pool = consts["pool"]
    spool = consts["spool"]
    apsum = consts["apsum"]
    DA = dglen + nheads
    nj = K[t]
    nch = -(-nj // R)
    ci0 = chunk_of[(t, 0)]

    sst = spool.tile([P, CMAX, R, 2 * P], bf16, tag="sst")
    nc.scalar.dma_start(
        out=sst[:, :nch, :, :].rearrange("p c r d -> p c (r d)"),
        in_=sstc_d[ci0:ci0 + nch, :, :, :].rearrange("c p r d -> p c (r d)"))

    for j0 in range(0, nj, R):
        r = min(R, nj - j0)
        ck = j0 // R
        g = getg(j0, r)
        ade = apsum.tile([P, R, nheads], f32, tag="ade")
        for ri in range(r):
            nc.tensor.matmul(
                ade[:, ri, :], lhsT=sst[:, ck, ri, :P],
                rhs=selfrow[:, dglen + nheads:dglen + 2 * nheads],
                start=True, stop=True)
        s = pool.tile([P, R, nheads], f32, tag="s")
        nc.vector.tensor_add(out=s[:, :r], in0=g[:, :r, dglen:dglen + nheads],
                             in1=ade[:, :r])
        e = pool.tile([P, R, nheads], f32, tag="e")
        nc.vector.scalar_tensor_tensor(
            out=e[:, :r], in0=s[:, :r], scalar=NEG, in1=s[:, :r],
            op0=MULT, op1=MAX)
        rhs = pool.tile([P, R, DA], bf16, tag="rhs")
        nc.scalar.activation(out=rhs[:, :r, dglen:], in_=e[:, :r], func=Exp)
        nc.vector.tensor_tensor(
            out=rhs[:, :r, :dglen].rearrange("p r (h c) -> p r h c", h=nheads),
            in0=g[:, :r, :dglen].rearrange("p r (h c) -> p r h c", h=nheads),
            in1=rhs[:, :r, dglen:].rearrange("p r (h o) -> p r h o", o=1)
                .to_broadcast([P, r, nheads, hw]),
            op=MULT)
        for ri in range(r):
            nc.tensor.matmul(
                agg[:], lhsT=sst[:, ck, ri, P:], rhs=rhs[:, ri, :],
                start=(j0 == 0 and ri == 0), stop=(j0 + ri == nj - 1))


def _build_program(meta):
    T, SH, SH2, NPAD = meta["T"], meta["SH"], meta["SH2"], meta["NPAD"]
    CT, CT0, NCHT = meta["CT"], meta["CT0"], meta["NCHT"]
    K, K0, offs, offs0 = meta["K"], meta["K0"], meta["offs"], meta["offs0"]
    chunk_of = meta["chunk_of"]
    K0MAX = max(max(K0), 1)

    nc = bacc.Bacc("TRN2", target_bir_lowering=False, debug=False, num_devices=W)

    xetc_d = nc.dram_tensor("XETC", [NCHT, P, 2, R, P], bf16, kind="ExternalInput")
    xts_d = nc.dram_tensor("xTs", [F_IN, SH], bf16, kind="ExternalInput")
    w1_d = nc.dram_tensor("W1", [F_IN, F_IN], f32, kind="ExternalInput")
    asrc_d = nc.dram_tensor("asrc", [1, F_IN], f32, kind="ExternalInput")
    adstv_d = nc.dram_tensor("adstv", [1, F_IN], f32, kind="ExternalInput")
    b1_d = nc.dram_tensor("b1", [1, F_IN], f32, kind="ExternalInput")
    w2_d = nc.dram_tensor("W2", [F_IN, NCLS], f32, kind="ExternalInput")
    a2s_d = nc.dram_tensor("a2s", [1, NCLS], f32, kind="ExternalInput")
    a2d_d = nc.dram_tensor("a2d", [1, NCLS], f32, kind="ExternalInput")
    b2_d = nc.dram_tensor("b2", [1, NCLS], f32, kind="ExternalInput")
    srcg2_d = nc.dram_tensor("srcg2", [P, CT], i32, kind="ExternalInput")
    sstc_d = nc.dram_tensor("sstc", [NCHT, P, R, 2 * P], bf16, kind="ExternalInput")
    out_d = nc.dram_tensor("out", [SH, NCLS], f32, kind="ExternalOutput")

    HAUGs = nc.dram_tensor("HAUGs", [SH, D1], bf16)
    Z = nc.dram_tensor("Z", [SH, F_IN], bf16)
    GE0 = nc.dram_tensor("GE0", [P, max(CT0, 1), D2], bf16)

    with tile.TileContext(nc) as tc:
        with contextlib.ExitStack() as top:
            cpool = top.enter_context(tc.tile_pool(name="const", bufs=1))
            dram = top.enter_context(tc.tile_pool(name="dram", bufs=1, space="DRAM"))

            srcb2 = cpool.tile([P, CT], i32)
            nc.sync.dma_start(out=srcb2[:], in_=srcg2_d[:])

            rhs1 = [cpool.tile([P, D1], bf16, name=f"rhs1_{k}") for k in range(2)]
            rhs2 = [cpool.tile([P, D2], bf16, name=f"rhs2_{k}") for k in range(2)]
            b1_b = cpool.tile([P, F_IN], f32)
            b2p_b = cpool.tile([P, D2], f32)

            # ---- setup: broadcast rows + fold attention vectors into rhs ----
            with contextlib.ExitStack() as su:
                spool = su.enter_context(tc.tile_pool(name="setup", bufs=1))
                spsum = su.enter_context(tc.tile_pool(name="setup_ps", bufs=1, space="PSUM"))
                ones = spool.tile([1, P], f32)
                nc.vector.memset(ones[:], 1.0)

                def bcast(dram_ap, width, out_ap):
                    ps = spsum.tile([P, width], f32, tag="bps")
                    row = spool.tile([1, width], f32, tag="brow")
                    nc.sync.dma_start(out=row[:], in_=dram_ap)
                    nc.tensor.matmul(ps[:], lhsT=ones[:], rhs=row[:], start=True, stop=True)
                    nc.vector.tensor_copy(out=out_ap, in_=ps[:])

                asrc_b = spool.tile([P, F_IN], f32)
                bcast(asrc_d[:], F_IN, asrc_b[:])
                adst_b = spool.tile([P, F_IN], f32)
                bcast(adstv_d[:], F_IN, adst_b[:])
                bcast(b1_d[:], F_IN, b1_b[:])
                a2s_b = spool.tile([P, NCLS], f32)
                bcast(a2s_d[:], NCLS, a2s_b[:])
                a2d_b = spool.tile([P, NCLS], f32)
                bcast(a2d_d[:], NCLS, a2d_b[:])
                nc.vector.memset(b2p_b[:], 0.0)
                bcast(b2_d[:], NCLS, b2p_b[:, :NCLS])

                for k in range(2):
                    w1sb = spool.tile([P, F_IN], f32, tag="w1sb")
                    nc.sync.dma_start(out=w1sb[:], in_=w1_d[k * P:(k + 1) * P, :])
                    nc.vector.tensor_copy(out=rhs1[k][:, :F_IN], in_=w1sb[:])
                    for vec_b, col in ((asrc_b, F_IN), (adst_b, F_IN + H)):
                        tmp = spool.tile([P, F_IN], f32, tag="tmp")
                        nc.vector.tensor_mul(out=tmp[:], in0=w1sb[:], in1=vec_b[:])
                        vred = spool.tile([P, H], f32, tag="vred")
                        nc.vector.tensor_reduce(
                            out=vred[:], in_=tmp[:].rearrange("p (h c) -> p h c", h=H),
                            axis=mybir.AxisListType.X, op=ADD)
                        nc.vector.tensor_copy(out=rhs1[k][:, col:col + H], in_=vred[:])

                    w2sb = spool.tile([P, NCLS], f32, tag="w2sb")
                    nc.sync.dma_start(out=w2sb[:], in_=w2_d[k * P:(k + 1) * P, :])
                    nc.vector.tensor_copy(out=rhs2[k][:, :NCLS], in_=w2sb[:])
                    for vec_b, cs in ((a2s_b, slice(NCLS, NCLS + 1)),
                                      (a2d_b, slice(NCLS + 1, D2))):
                        tmp2 = spool.tile([P, NCLS], f32, tag="tmp2")
                        nc.vector.tensor_mul(out=tmp2[:], in0=w2sb[:], in1=vec_b[:])
                        vred2 = spool.tile([P, 1], f32, tag="vred2")
                        nc.vector.tensor_reduce(
                            out=vred2[:], in_=tmp2[:].rearrange("p (o c) -> p o c", o=1),
                            axis=mybir.AxisListType.X, op=ADD)
                        n_rep = cs.stop - cs.start
                        nc.vector.tensor_copy(
                            out=rhs2[k][:, cs], in_=vred2[:].to_broadcast([P, n_rep]))

            # ---- Phase A: HAUGs (bias-free) for own dst shard ----
            with contextlib.ExitStack() as pa:
                apool = pa.enter_context(tc.tile_pool(name="pa", bufs=4))
                apsum = pa.enter_context(tc.tile_pool(name="pa_ps", bufs=2, space="PSUM"))
                BS = min(14, T)
                for b0 in range(0, T, BS):
                    nb = min(BS, T - b0)
                    xt = [apool.tile([P, BS * P], bf16, tag=f"xt{k}", name=f"xt{k}")
                          for k in range(2)]
                    for k in range(2):
                        nc.sync.dma_start(
                            out=xt[k][:, :nb * P],
                            in_=xts_d[k * P:(k + 1) * P, b0 * P:(b0 + nb) * P])
                    hsb = apool.tile([P, BS, D1], bf16, tag="hsb")
                    for nt in range(nb):
                        ps = apsum.tile([P, D1], f32, tag="aps")
                        for k in range(2):
                            nc.tensor.matmul(
                                ps[:], lhsT=xt[k][:, nt * P:(nt + 1) * P], rhs=rhs1[k][:],
                                start=(k == 0), stop=(k == 1))
                        nc.scalar.activation(out=hsb[:, nt, :], in_=ps[:],
                                             func=Copy)
                    row0 = b0 * P
                    nc.scalar.dma_start(
                        out=HAUGs[row0:row0 + nb * P, :].rearrange("(a p) d -> p a d", p=P),
                        in_=hsb[:, :nb, :])

            cpool2 = top.enter_context(tc.tile_pool(name="pc", bufs=3))
            cpsum = top.enter_context(tc.tile_pool(name="pc_ps", bufs=1, space="PSUM"))
            T2L = dram.tile([SH, D2], bf16, name="T2L")
            ep_pool = top.enter_context(tc.tile_pool(name="ep", bufs=8))
            ep_spool = top.enter_context(tc.tile_pool(name="ep_s", bufs=3))
            ep_fpool = top.enter_context(tc.tile_pool(name="ep_f", bufs=3))
            ep_psum = top.enter_context(tc.tile_pool(name="ep_p", bufs=2, space="PSUM"))
            ep_apsum = top.enter_context(tc.tile_pool(name="ep_a", bufs=2, space="PSUM"))
            ep_hpsum = top.enter_context(tc.tile_pool(name="ep_h", bufs=3, space="PSUM"))
            consts = dict(sstc_d=sstc_d, pool=ep_pool, spool=ep_spool,
                          fpool=ep_fpool, psum=ep_psum, apsum=ep_apsum)

            # T2 AllGather halves (layer-2 gather tables)
            T2F0 = dram.tile([W * SH2, D2], bf16, name="T2F0", addr_space="Shared")
            T2F1 = dram.tile([W * SH2, D2], bf16, name="T2F1", addr_space="Shared")

            # ---- Phase B: layer-1 edge phase (dense) -> Z, T2L ----
            def flush1(t, agg, selfrow, fpool):
                es = fpool.tile([P, H], f32, tag="es")
                nc.vector.tensor_add(out=es[:], in0=selfrow[:, F_IN:F_IN + H],
                                     in1=selfrow[:, F_IN + H:])
                nc.vector.scalar_tensor_tensor(
                    out=es[:], in0=es[:], scalar=NEG, in1=es[:], op0=MULT, op1=MAX)
                exs = fpool.tile([P, H], f32, tag="exs")
                nc.scalar.activation(out=exs[:], in_=es[:], func=Exp)
                selfsc = fpool.tile([P, F_IN], f32, tag="selfsc")
                nc.vector.tensor_tensor(
                    out=selfsc[:].rearrange("p (h c) -> p h c", h=H),
                    in0=selfrow[:, :F_IN].rearrange("p (h c) -> p h c", h=H),
                    in1=exs[:].rearrange("p (h o) -> p h o", o=1).to_broadcast([P, H, C]),
                    op=MULT)
                numer = fpool.tile([P, F_IN], f32, tag="numer")
                nc.vector.tensor_add(out=numer[:], in0=selfsc[:], in1=agg[:, :F_IN])
                dinv = fpool.tile([P, H], f32, tag="dinv")
                nc.vector.tensor_add(out=dinv[:], in0=exs[:], in1=agg[:, F_IN:])
                nc.vector.tensor_scalar_add(out=dinv[:], in0=dinv[:], scalar1=1e-16)
                nc.vector.reciprocal(out=dinv[:], in_=dinv[:])
                o = fpool.tile([P, F_IN], f32, tag="o")
                nc.vector.tensor_tensor(
                    out=o[:].rearrange("p (h c) -> p h c", h=H),
                    in0=numer[:].rearrange("p (h c) -> p h c", h=H),
                    in1=dinv[:].rearrange("p (h o) -> p h o", o=1).to_broadcast([P, H, C]),
                    op=MULT)
                nc.vector.tensor_add(out=o[:], in0=o[:], in1=b1_b[:])
                mmin = fpool.tile([P, F_IN], f32, tag="mmin")
                nc.vector.tensor_scalar_min(out=mmin[:], in0=o[:], scalar1=0.0)
                ex = fpool.tile([P, F_IN], f32, tag="ex")
                nc.scalar.activation(out=ex[:], in_=mmin[:], func=Exp)
                rel = fpool.tile([P, F_IN], f32, tag="rel")
                nc.vector.tensor_scalar_max(out=rel[:], in0=o[:], scalar1=0.0)
                z = fpool.tile([P, F_IN], bf16, tag="z")
                nc.vector.scalar_tensor_tensor(
                    out=z[:], in0=ex[:], scalar=-1.0, in1=rel[:], op0=ADD, op1=ADD)
                nc.sync.dma_start(out=Z[t * P:(t + 1) * P, :], in_=z[:])
                # Phase C for this tile, overlapped with the edge phase
                zt = [cpool2.tile([P, P], bf16, tag=f"zt{k}", name=f"zt{k}")
                      for k in range(2)]
                for k in range(2):
                    nc.sync.dma_start(
                        out=zt[k][:], in_=Z[t * P:(t + 1) * P, k * P:(k + 1) * P],
                        transpose=True)
                ps2 = cpsum.tile([P, D2], f32, tag="cps")
                for k in range(2):
                    nc.tensor.matmul(ps2[:], lhsT=zt[k][:], rhs=rhs2[k][:],
                                     start=(k == 0), stop=(k == 1))
                t2sb = cpool2.tile([P, D2], bf16, tag="t2sb")
                nc.vector.tensor_add(out=t2sb[:], in0=ps2[:], in1=b2p_b[:])
                nc.sync.dma_start(out=T2L[t * P:(t + 1) * P, :], in_=t2sb[:])

            t2f0_ap = T2F0.tensor.ap()
            t2f1_ap = T2F1.tensor.ap()

            def stage0(t, stpool):
                """Gather this tile's bucket-0 T2 rows and stage them to DRAM
                (runs on gpsimd+sync, concurrent with phase-B compute)."""
                k0 = K0[t]
                if k0 == 0:
                    return
                b0 = int(offs0[t])
                base = int(offs[t])
                ring = stpool.tile([P, K0MAX, D2], bf16, tag="st")
                for j in range(k0):
                    c1 = base + j
                    nc.gpsimd.indirect_dma_start(
                        out=ring[:, j, :], out_offset=None, in_=t2f0_ap[:],
                        in_offset=bass.IndirectOffsetOnAxis(
                            ap=srcb2[:, c1:c1 + 1], axis=0),
                    )
                nc.sync.dma_start(out=GE0[:, b0:b0 + k0, :], in_=ring[:, :k0, :])

            def phase_b_tile(t, bpool):
                agg = ep_psum.tile([P, F_IN + H], f32, tag="agg")
                selfrow = ep_fpool.tile([P, D1], bf16, tag="selfrow")
                nc.sync.dma_start(out=selfrow[:],
                                  in_=HAUGs[t * P:(t + 1) * P, :])
                nch = -(-K[t] // R)
                ci0 = chunk_of[(t, 0)]
                xe_cache = {}

                def getg1(j0, r, t=t):
                    ck = j0 // R
                    pb = (ck // 2) * 2      # load xe two chunks per DMA
                    if pb not in xe_cache:
                        np_ = min(2, nch - pb)
                        xe = bpool.tile([P, 2, 2, R, P], bf16, tag="xe")
                        nc.sync.dma_start(
                            out=xe[:, :np_, :, :, :]
                                .rearrange("p c k r e -> p c (k r e)"),
                            in_=xetc_d[ci0 + pb:ci0 + pb + np_, :, :, :, :]
                                .rearrange("c p k r e -> p c (k r e)"))
                        xe_cache[pb] = xe
                    xe = xe_cache[pb]
                    hb = bpool.tile([P, R, D1], bf16, tag="hb")
                    for ri in range(r):
                        hp = ep_hpsum.tile([P, D1], f32, tag="hp")
                        for k in range(2):
                            nc.tensor.matmul(
                                hp[:], lhsT=xe[:, ck - pb, k, ri, :], rhs=rhs1[k][:],
                                start=(k == 0), stop=(k == 1))
                        nc.scalar.activation(out=hb[:, ri, :], in_=hp[:],
                                             func=Copy)
                    return hb[:]

                _edge_chunks(nc, meta, consts, t=t, selfrow=selfrow,
                             getg=getg1, dglen=F_IN, nheads=H, hw=C, agg=agg)
                flush1(t, agg, selfrow, ep_fpool)

            TH = (T + 1) // 2
            with contextlib.ExitStack() as pb:
                bpool = pb.enter_context(tc.tile_pool(name="pb", bufs=8))
                stpool = pb.enter_context(tc.tile_pool(name="pst", bufs=6))
                for t in range(TH):
                    phase_b_tile(t, bpool)
                # first-half T2 rows are final -> AllGather half 0
                nc.gpsimd.collective_compute(
                    "AllGather", mybir.AluOpType.bypass,
                    replica_groups=[list(range(W))],
                    ins=[T2L[0:SH2, :]], outs=[T2F0.opt()])
                # second half of phase B, with bucket-0 layer-2 gathers
                # (2 staging tiles per compute tile) interleaved
                st_t = 0
                for t in range(TH, T):
                    phase_b_tile(t, bpool)
                    for _ in range(2):
                        if st_t < T:
                            stage0(st_t, stpool)
                            st_t += 1
                while st_t < T:
                    stage0(st_t, stpool)
                    st_t += 1
                nc.gpsimd.collective_compute(
                    "AllGather", mybir.AluOpType.bypass,
                    replica_groups=[list(range(W))],
                    ins=[T2L[SH2:SH, :]], outs=[T2F1.opt()])

            # ---- Phase D: layer-2 edge phase -> out ----
            def flush2(t, agg, selfrow, fpool):
                es = fpool.tile([P, 1], f32, tag="es2")
                nc.vector.tensor_add(out=es[:], in0=selfrow[:, NCLS:NCLS + 1],
                                     in1=selfrow[:, NCLS + 1:NCLS + 2])
                nc.vector.scalar_tensor_tensor(
                    out=es[:], in0=es[:], scalar=NEG, in1=es[:], op0=MULT, op1=MAX)
                exs = fpool.tile([P, 1], f32, tag="exs2")
                nc.scalar.activation(out=exs[:], in_=es[:], func=Exp)
                selfsc = fpool.tile([P, NCLS], f32, tag="selfsc2")
                nc.vector.tensor_tensor(
                    out=selfsc[:], in0=selfrow[:, :NCLS],
                    in1=exs[:].to_broadcast([P, NCLS]), op=MULT)
                numer = fpool.tile([P, NCLS], f32, tag="numer2")
                nc.vector.tensor_add(out=numer[:], in0=selfsc[:], in1=agg[:, :NCLS])
                dinv = fpool.tile([P, 1], f32, tag="dinv2")
                nc.vector.tensor_add(out=dinv[:], in0=exs[:], in1=agg[:, NCLS:])
                nc.vector.tensor_scalar_add(out=dinv[:], in0=dinv[:], scalar1=1e-16)
                nc.vector.reciprocal(out=dinv[:], in_=dinv[:])
                o = fpool.tile([P, NCLS], f32, tag="o2")
                nc.vector.tensor_tensor(
                    out=o[:], in0=numer[:], in1=dinv[:].to_broadcast([P, NCLS]), op=MULT)
                nc.sync.dma_start(out=out_d[t * P:(t + 1) * P, :], in_=o[:])

            with contextlib.ExitStack() as pd:
                dpool = pd.enter_context(tc.tile_pool(name="pd", bufs=8))
                for t in range(T):
                    agg = ep_psum.tile([P, NCLS + 1], f32, tag="agg")
                    base = int(offs[t])
                    b0 = int(offs0[t])
                    k0 = K0[t]
                    selfrow = ep_fpool.tile([P, D2], bf16, tag="selfrow2")
                    nc.sync.dma_start(out=selfrow[:],
                                      in_=T2L[t * P:(t + 1) * P, :])

                    def getg2(j0, r, base=base, b0=b0, k0=k0):
                        g = dpool.tile([P, R, D2], bf16, tag="g2")
                        nb0 = max(0, min(k0 - j0, r))
                        if nb0 > 0:         # staged bucket-0 rows
                            nc.sync.dma_start(
                                out=g[:, :nb0, :],
                                in_=GE0[:, b0 + j0:b0 + j0 + nb0, :])
                        for ri in range(nb0, r):   # bucket-1 gathers
                            c1 = base + j0 + ri
                            nc.gpsimd.indirect_dma_start(
                                out=g[:, ri, :], out_offset=None, in_=t2f1_ap[:],
                                in_offset=bass.IndirectOffsetOnAxis(
                                    ap=srcb2[:, c1:c1 + 1], axis=0),
                            )
                        return g[:]

                    _edge_chunks(nc, meta, consts, t=t, selfrow=selfrow,
                                 getg=getg2, dglen=NCLS, nheads=1, hw=NCLS,
                                 agg=agg)
                    flush2(t, agg, selfrow, ep_fpool)

    nc.compile()
    return nc


def kernel(**inputs):
    x = np.asarray(inputs["x"], np.float32)
    edge_index = np.asarray(inputs["edge_index"])
    meta, arrays = _host_prep(x, edge_index)

    key = (meta["N"], meta["E"], meta["K"], meta["K0"])
    if key not in _CACHE:
        _CACHE[key] = _build_program(meta)
    nc = _CACHE[key]

    common = {
        "W1": np.asarray(inputs["W1"], np.float32),
        "asrc": np.asarray(inputs["att_src1"], np.float32).reshape(1, -1),
        "adstv": np.asarray(inputs["att_dst1"], np.float32).reshape(1, -1),
        "b1": np.asarray(inputs["bias1"], np.float32).reshape(1, -1),
        "W2": np.asarray(inputs["W2"], np.float32),
        "a2s": np.asarray(inputs["att_src2"], np.float32).reshape(1, -1),
        "a2d": np.asarray(inputs["att_dst2"], np.float32).reshape(1, -1),
        "b2": np.asarray(inputs["bias2"], np.float32).reshape(1, -1),
    }
    in_maps = []
    for c in range(W):
        m = dict(common)
        m["XETC"] = arrays["build_xetc"](c)
        m["xTs"] = arrays["xts"][c]
        m["srcg2"] = arrays["t2row"][c]
        m["sstc"] = _make_sstc(arrays["dstl_arr"][c], meta)
        in_maps.append(m)

    res = run_bass_kernel_spmd(nc, in_maps, core_ids=list(range(W)), trace=TRACE)
    kernel.last_results = res

    N = meta["N"]
    out = np.concatenate([res.results[c]["out"] for c in range(W)], axis=0)
    return np.ascontiguousarray(out[:N])


# revision 21
# speedup vs baseline: 1.0099x; 1.0099x over previous
"""Two-layer GAT (PyG GATConv semantics) on 8 TRN2 NeuronCores.

Strategy (edge/graph parallel, v5):
  - Host (index manipulation only): sort non-self-loop edges by dst, shard
    dst nodes contiguously across the 8 cores, pad each dst tile's edge
    list to a multiple of 128, and precompute each 128-edge tile's 0/1
    incidence matrices S (dst x edge) and S^T as bf16 (pure index data).
    The host also pre-permutes x rows into per-edge-tile transposed blocks
    XET[c] = x[src[tile c]].T (a gather/reshape of the input - no
    arithmetic) so layer 1 needs NO device-side indirect DMA.
  - Phase A (per core, own dst shard): HAUGs[v] = [x@W1 | a_src | a_dst]
    (272 cols, bf16); bias b1 is applied post-softmax in the flush.
  - Phase B (edge phase, layer 1): per 128-edge tile, he = XET^T @ W1aug
    on the tensor engine; alpha_dst per edge via a small matmul
    S^T @ a_dst_tile; exp(leakyrelu(asrc+adst)); one incidence matmul
    accumulates [sum exp*h | sum exp] in PSUM.  Self-loops are added
    densely in the flush.  Normalize, +b1, ELU, store z; layer-2 rows
    T2[v] = [z@W2 + b2 | a2_src | a2_dst] computed inline (Phase C).
  - The T2 AllGather is split into NQ=4 quarters; layer-2 edges are
    bucketed by src quarter.  Buckets 0..2 are indirect-gathered and
    staged to DRAM concurrently with phase-B compute (GpSimd is otherwise
    idle there), so only bucket-3 gathers remain serialized in Phase D.
  - Phase D: layer-2 edge phase (heads=1) -> output shard.

All floating-point math happens on device; the host only reorders
indices/rows and pads/reshapes layouts.
"""

import contextlib
from collections import deque

import numpy as np

import concourse.bass as bass
import concourse.bacc as bacc
import concourse.mybir as mybir
import concourse.tile as tile
from concourse.bass_utils import run_bass_kernel_spmd

# ---- fixed problem hyperparameters (from the nn.Module) ----
F_IN = 256
H = 8
C = 32
NCLS = 40
NEG = 0.2

W = 8               # cores
P = 128             # partitions
D1 = F_IN + 2 * H   # HAUG row: [h (256) | a_src (8) | a_dst (8)] = 272
D2 = NCLS + 8       # T2 row: [h2+b2 (40) | a2_src (1) | a2_dst x7 (41:48)]
R = 4               # edge tiles batched per compute group
NQ = 4              # AllGather split (src-quarter buckets)

f32 = mybir.dt.float32
bf16 = mybir.dt.bfloat16
i32 = mybir.dt.int32
BF_NP = mybir.dt.np(bf16)

Exp = mybir.ActivationFunctionType.Exp
Copy = mybir.ActivationFunctionType.Copy
ADD = mybir.AluOpType.add
MULT = mybir.AluOpType.mult
MAX = mybir.AluOpType.max

TRACE = False       # set by test harness for profiling runs
_CACHE = {}


def _host_prep(x, edge_index):
    """Index-only preprocessing. Returns (meta, per-core arrays)."""
    N = x.shape[0]
    E = edge_index.shape[1]
    src_a = np.asarray(edge_index[0], np.int64)
    dst_a = np.asarray(edge_index[1], np.int64)
    order = np.argsort(dst_a, kind="stable")
    src_s = src_a[order].astype(np.int64)
    dst_s = dst_a[order].astype(np.int64)

    nt_real = -(-N // P)
    T = -(-nt_real // W)
    SH = T * P
    NPAD = W * SH

    nq = min(NQ, T)
    bs, rem = divmod(T, nq)
    sizes = [bs + (1 if q < rem else 0) for q in range(nq)]
    tqb = np.zeros(nq + 1, np.int64)
    tqb[1:] = np.cumsum(sizes)           # quarter tile boundaries

    # bucket edges by (core, dst tile, src quarter) for the split AllGather
    gt = dst_s // P
    l_all = src_s % SH
    qq = np.searchsorted(tqb[1:], l_all // P, side="right")
    key = gt * nq + qq
    cnt = np.bincount(key, minlength=W * T * nq).reshape(W, T, nq)
    Kq = -(-cnt.max(axis=0) // P)        # [T, nq]
    Kq[:, 0] = np.maximum(Kq[:, 0], 1)
    K = Kq.sum(axis=1)
    cumKq = np.zeros((T, nq + 1), np.int64)
    cumKq[:, 1:] = np.cumsum(Kq, axis=1)
    offs = np.zeros(T + 1, np.int64)
    offs[1:] = np.cumsum(K)
    CT = int(offs[-1])

    sort2 = np.argsort(key, kind="stable")
    src_s = src_s[sort2]
    dst_s = dst_s[sort2]
    gt = gt[sort2]
    qq = qq[sort2]
    key = key[sort2]
    start_k = np.zeros(W * T * nq + 1, np.int64)
    start_k[1:] = np.cumsum(cnt.reshape(-1))
    rank = np.arange(len(dst_s)) - start_k[key]
    c_all = gt // T
    t_all = gt % T
    col = offs[t_all] + cumKq[t_all, qq] + rank // P
    p_all = rank % P

    src_arr = np.zeros((W, P, CT), np.int64)
    dstl_arr = np.full((W, P, CT), -1, np.int16)
    src_arr[c_all, p_all, col] = src_s
    dstl_arr[c_all, p_all, col] = (dst_s % P).astype(np.int16)

    # layer-2 gather rows in the split-T2F layout:
    # node v = s*SH + l in quarter q -> T2Fq row s*LENq + (l - tqb[q]*P)
    s_of = src_arr // SH
    l_of = src_arr % SH
    q_of = np.searchsorted(tqb[1:], l_of // P, side="right")
    lenq = np.array([sizes[q] * P for q in range(nq)], np.int64)
    qst = tqb[:-1] * P
    t2row = (s_of * lenq[q_of] + (l_of - qst[q_of])).astype(np.int32)
    src_arr = src_arr.astype(np.int32)

    chunks = []                          # (t, base, j0, r)
    chunk_of = {}
    for t in range(T):
        for j0 in range(0, int(K[t]), R):
            r = min(R, int(K[t]) - j0)
            chunk_of[(t, j0)] = len(chunks)
            chunks.append((t, int(offs[t]), j0, r))
    NCHT = len(chunks)

    KS = Kq[:, :nq - 1].sum(axis=1)      # staged (non-final-bucket) cols
    goffs = np.zeros(T + 1, np.int64)
    goffs[1:] = np.cumsum(KS)
    CTG = int(goffs[-1])
    KQMAX = int(max(1, Kq[:, :max(nq - 1, 1)].max()))

    meta = dict(N=N, E=E, T=T, SH=SH, NPAD=NPAD, nq=nq,
                sizes=tuple(sizes), tqb=tuple(int(v) for v in tqb),
                K=tuple(int(k) for k in K),
                Kq=tuple(tuple(int(v) for v in row) for row in Kq),
                cumKq=tuple(tuple(int(v) for v in row) for row in cumKq),
                KS=tuple(int(v) for v in KS),
                offs=offs, goffs=goffs, CT=CT, CTG=CTG, NCHT=NCHT,
                chunks=tuple(chunks), chunk_of=chunk_of, KQMAX=KQMAX)

    xb = np.zeros((NPAD, F_IN), BF_NP)
    xb[:N] = np.asarray(x, np.float32).astype(BF_NP)

    def build_xetc(c):
        # [NCHT, 2, P(feat), R, P(edge)] chunk-blocked transposed x rows
        out = np.zeros((NCHT, 2, P, R, P), BF_NP)
        for ci, (t, base, j0, r) in enumerate(chunks):
            cols = src_arr[c][:, base + j0:base + j0 + r]     # [P(edge), r]
            blk = xb[cols]                                    # [Pe, r, 256]
            out[ci, :, :, :r, :] = (blk.transpose(2, 1, 0)    # [256, r, Pe]
                                    .reshape(2, P, r, P))
        return out

    xts = [np.ascontiguousarray(xb[c * SH:(c + 1) * SH].T) for c in range(W)]

    arrays = dict(src_arr=src_arr, dstl_arr=dstl_arr, t2row=t2row,
                  build_xetc=build_xetc, xts=xts)
    return meta, arrays


def _make_sstc(dstl_core, meta):
    """[P, CT] int16 dst-local (-1 pad) -> [NCHT, P, R, 256] bf16 [S | S^T]."""
    chunks = meta["chunks"]
    NCHT = meta["NCHT"]
    iota = np.arange(P, dtype=np.int16)
    out = np.zeros((NCHT, P, R, 2 * P), BF_NP)
    for ci, (t, base, j0, r) in enumerate(chunks):
        d = dstl_core[:, base + j0:base + j0 + r].T          # [r, 128e]
        S = (d[:, None, :] == iota[None, :, None])           # [r, d, e]
        out[ci, :, :r, :P] = S.transpose(1, 0, 2)
        out[ci, :, :r, P:] = S.transpose(2, 0, 1)
    return out


def _edge_chunks(nc, meta, consts, *, t, selfrow, getg, dglen, nheads, hw,
                 agg):
    """Shared per-chunk compute: attention weights + incidence aggregation."""
    K = meta["K"]
    chunk_of = meta["chunk_of"]
    sstc_d = consts["sstc_d"]
    pool = consts["pool"]
    spool = consts["spool"]
    apsum = consts["apsum"]
    DA = dglen + nheads
    nj = K[t]

    for j0 in range(0, nj, R):
        r = min(R, nj - j0)
        ci = chunk_of[(t, j0)]
        g = getg(j0, r)
        sst = spool.tile([P, R, 2 * P], bf16, tag="sst")
        nc.scalar.dma_start(out=sst[:, :r, :], in_=sstc_d[ci, :, :r, :])
        ade = apsum.tile([P, R, nheads], f32, tag="ade")
        for ri in range(r):
            nc.tensor.matmul(
                ade[:, ri, :], lhsT=sst[:, ri, :P],
                rhs=selfrow[:, dglen + nheads:dglen + 2 * nheads],
                start=True, stop=True)
        s = pool.tile([P, R, nheads], f32, tag="s")
        nc.vector.tensor_add(out=s[:, :r], in0=g[:, :r, dglen:dglen + nheads],
                             in1=ade[:, :r])
        e = pool.tile([P, R, nheads], f32, tag="e")
        nc.vector.scalar_tensor_tensor(
            out=e[:, :r], in0=s[:, :r], scalar=NEG, in1=s[:, :r],
            op0=MULT, op1=MAX)
        rhs = pool.tile([P, R, DA], bf16, tag="rhs")
        nc.scalar.activation(out=rhs[:, :r, dglen:], in_=e[:, :r], func=Exp)
        nc.vector.tensor_tensor(
            out=rhs[:, :r, :dglen].rearrange("p r (h c) -> p r h c", h=nheads),
            in0=g[:, :r, :dglen].rearrange("p r (h c) -> p r h c", h=nheads),
            in1=rhs[:, :r, dglen:].rearrange("p r (h o) -> p r h o", o=1)
                .to_broadcast([P, r, nheads, hw]),
            op=MULT)
        for ri in range(r):
            nc.tensor.matmul(
                agg[:], lhsT=sst[:, ri, P:], rhs=rhs[:, ri, :],
                start=(j0 == 0 and ri == 0), stop=(j0 + ri == nj - 1))


def _build_program(meta):
    T, SH, NPAD = meta["T"], meta["SH"], meta["NPAD"]
    CT, CTG, NCHT = meta["CT"], meta["CTG"], meta["NCHT"]
    nq, sizes, tqb = meta["nq"], meta["sizes"], meta["tqb"]
    K, Kq, cumKq, KS = meta["K"], meta["Kq"], meta["cumKq"], meta["KS"]
    offs, goffs = meta["offs"], meta["goffs"]
    chunk_of = meta["chunk_of"]
    KQMAX = meta["KQMAX"]

    nc = bacc.Bacc("TRN2", target_bir_lowering=False, debug=False, num_devices=W)

    xetc_d = nc.dram_tensor("XETC", [NCHT, 2, P, R, P], bf16, kind="ExternalInput")
    xts_d = nc.dram_tensor("xTs", [F_IN, SH], bf16, kind="ExternalInput")
    w1_d = nc.dram_tensor("W1", [F_IN, F_IN], f32, kind="ExternalInput")
    asrc_d = nc.dram_tensor("asrc", [1, F_IN], f32, kind="ExternalInput")
    adstv_d = nc.dram_tensor("adstv", [1, F_IN], f32, kind="ExternalInput")
    b1_d = nc.dram_tensor("b1", [1, F_IN], f32, kind="ExternalInput")
    w2_d = nc.dram_tensor("W2", [F_IN, NCLS], f32, kind="ExternalInput")
    a2s_d = nc.dram_tensor("a2s", [1, NCLS], f32, kind="ExternalInput")
    a2d_d = nc.dram_tensor("a2d", [1, NCLS], f32, kind="ExternalInput")
    b2_d = nc.dram_tensor("b2", [1, NCLS], f32, kind="ExternalInput")
    srcg2_d = nc.dram_tensor("srcg2", [P, CT], i32, kind="ExternalInput")
    sstc_d = nc.dram_tensor("sstc", [NCHT, P, R, 2 * P], bf16, kind="ExternalInput")
    out_d = nc.dram_tensor("out", [SH, NCLS], f32, kind="ExternalOutput")

    HAUGs = nc.dram_tensor("HAUGs", [SH, D1], bf16)
    Z = nc.dram_tensor("Z", [SH, F_IN], bf16)
    GE0 = nc.dram_tensor("GE0", [P, max(CTG, 1), D2], bf16)

    with tile.TileContext(nc) as tc:
        with contextlib.ExitStack() as top:
            cpool = top.enter_context(tc.tile_pool(name="const", bufs=1))
            dram = top.enter_context(tc.tile_pool(name="dram", bufs=1, space="DRAM"))

            srcb2 = cpool.tile([P, CT], i32)
            nc.sync.dma_start(out=srcb2[:], in_=srcg2_d[:])

            rhs1 = [cpool.tile([P, D1], bf16, name=f"rhs1_{k}") for k in range(2)]
            rhs2 = [cpool.tile([P, D2], bf16, name=f"rhs2_{k}") for k in range(2)]
            b1_b = cpool.tile([P, F_IN], f32)
            b2p_b = cpool.tile([P, D2], f32)

            # ---- setup: broadcast rows + fold attention vectors into rhs ----
            with contextlib.ExitStack() as su:
                spool = su.enter_context(tc.tile_pool(name="setup", bufs=1))
                spsum = su.enter_context(tc.tile_pool(name="setup_ps", bufs=1, space="PSUM"))
                ones = spool.tile([1, P], f32)
                nc.vector.memset(ones[:], 1.0)

                def bcast(dram_ap, width, out_ap):
                    ps = spsum.tile([P, width], f32, tag="bps")
                    row = spool.tile([1, width], f32, tag="brow")
                    nc.sync.dma_start(out=row[:], in_=dram_ap)
                    nc.tensor.matmul(ps[:], lhsT=ones[:], rhs=row[:], start=True, stop=True)
                    nc.vector.tensor_copy(out=out_ap, in_=ps[:])

                asrc_b = spool.tile([P, F_IN], f32)
                bcast(asrc_d[:], F_IN, asrc_b[:])
                adst_b = spool.tile([P, F_IN], f32)
                bcast(adstv_d[:], F_IN, adst_b[:])
                bcast(b1_d[:], F_IN, b1_b[:])
                a2s_b = spool.tile([P, NCLS], f32)
                bcast(a2s_d[:], NCLS, a2s_b[:])
                a2d_b = spool.tile([P, NCLS], f32)
                bcast(a2d_d[:], NCLS, a2d_b[:])
                nc.vector.memset(b2p_b[:], 0.0)
                bcast(b2_d[:], NCLS, b2p_b[:, :NCLS])

                for k in range(2):
                    w1sb = spool.tile([P, F_IN], f32, tag="w1sb")
                    nc.sync.dma_start(out=w1sb[:], in_=w1_d[k * P:(k + 1) * P, :])
                    nc.vector.tensor_copy(out=rhs1[k][:, :F_IN], in_=w1sb[:])
                    for vec_b, colx in ((asrc_b, F_IN), (adst_b, F_IN + H)):
                        tmp = spool.tile([P, F_IN], f32, tag="tmp")
                        nc.vector.tensor_mul(out=tmp[:], in0=w1sb[:], in1=vec_b[:])
                        vred = spool.tile([P, H], f32, tag="vred")
                        nc.vector.tensor_reduce(
                            out=vred[:], in_=tmp[:].rearrange("p (h c) -> p h c", h=H),
                            axis=mybir.AxisListType.X, op=ADD)
                        nc.vector.tensor_copy(out=rhs1[k][:, colx:colx + H], in_=vred[:])

                    w2sb = spool.tile([P, NCLS], f32, tag="w2sb")
                    nc.sync.dma_start(out=w2sb[:], in_=w2_d[k * P:(k + 1) * P, :])
                    nc.vector.tensor_copy(out=rhs2[k][:, :NCLS], in_=w2sb[:])
                    for vec_b, cs in ((a2s_b, slice(NCLS, NCLS + 1)),
                                      (a2d_b, slice(NCLS + 1, D2))):
                        tmp2 = spool.tile([P, NCLS], f32, tag="tmp2")
                        nc.vector.tensor_mul(out=tmp2[:], in0=w2sb[:], in1=vec_b[:])
                        vred2 = spool.tile([P, 1], f32, tag="vred2")
                        nc.vector.tensor_reduce(
                            out=vred2[:], in_=tmp2[:].rearrange("p (o c) -> p o c", o=1),
                            axis=mybir.AxisListType.X, op=ADD)
                        n_rep = cs.stop - cs.start
                        nc.vector.tensor_copy(
                            out=rhs2[k][:, cs], in_=vred2[:].to_broadcast([P, n_rep]))

            # ---- Phase A: HAUGs (bias-free) for own dst shard ----
            with contextlib.ExitStack() as pa:
                apool = pa.enter_context(tc.tile_pool(name="pa", bufs=4))
                apsum = pa.enter_context(tc.tile_pool(name="pa_ps", bufs=2, space="PSUM"))
                BS = min(14, T)
                for b0 in range(0, T, BS):
                    nb = min(BS, T - b0)
                    xt = [apool.tile([P, BS * P], bf16, tag=f"xt{k}", name=f"xt{k}")
                          for k in range(2)]
                    for k in range(2):
                        nc.sync.dma_start(
                            out=xt[k][:, :nb * P],
                            in_=xts_d[k * P:(k + 1) * P, b0 * P:(b0 + nb) * P])
                    hsb = apool.tile([P, BS, D1], bf16, tag="hsb")
                    for nt in range(nb):
                        ps = apsum.tile([P, D1], f32, tag="aps")
                        for k in range(2):
                            nc.tensor.matmul(
                                ps[:], lhsT=xt[k][:, nt * P:(nt + 1) * P], rhs=rhs1[k][:],
                                start=(k == 0), stop=(k == 1))
                        nc.scalar.activation(out=hsb[:, nt, :], in_=ps[:],
                                             func=Copy)
                    row0 = b0 * P
                    nc.scalar.dma_start(
                        out=HAUGs[row0:row0 + nb * P, :].rearrange("(a p) d -> p a d", p=P),
                        in_=hsb[:, :nb, :])

            cpool2 = top.enter_context(tc.tile_pool(name="pc", bufs=3))
            cpsum = top.enter_context(tc.tile_pool(name="pc_ps", bufs=1, space="PSUM"))
            T2L = dram.tile([SH, D2], bf16, name="T2L")
            ep_pool = top.enter_context(tc.tile_pool(name="ep", bufs=8))
            ep_spool = top.enter_context(tc.tile_pool(name="ep_s", bufs=6))
            ep_fpool = top.enter_context(tc.tile_pool(name="ep_f", bufs=3))
            ep_psum = top.enter_context(tc.tile_pool(name="ep_p", bufs=2, space="PSUM"))
            ep_apsum = top.enter_context(tc.tile_pool(name="ep_a", bufs=2, space="PSUM"))
            ep_hpsum = top.enter_context(tc.tile_pool(name="ep_h", bufs=3, space="PSUM"))
            consts = dict(sstc_d=sstc_d, pool=ep_pool, spool=ep_spool,
                          fpool=ep_fpool, psum=ep_psum, apsum=ep_apsum)

            # per-quarter T2 AllGather outputs (layer-2 gather tables)
            t2f = [dram.tile([W * sizes[q] * P, D2], bf16, name=f"T2F{q}",
                             addr_space="Shared") for q in range(nq)]

            # ---- Phase B: layer-1 edge phase (dense) -> Z, T2L ----
            def flush1(t, agg, selfrow, fpool):
                es = fpool.tile([P, H], f32, tag="es")
                nc.vector.tensor_add(out=es[:], in0=selfrow[:, F_IN:F_IN + H],
                                     in1=selfrow[:, F_IN + H:])
                nc.vector.scalar_tensor_tensor(
                    out=es[:], in0=es[:], scalar=NEG, in1=es[:], op0=MULT, op1=MAX)
                exs = fpool.tile([P, H], f32, tag="exs")
                nc.scalar.activation(out=exs[:], in_=es[:], func=Exp)
                selfsc = fpool.tile([P, F_IN], f32, tag="selfsc")
                nc.vector.tensor_tensor(
                    out=selfsc[:].rearrange("p (h c) -> p h c", h=H),
                    in0=selfrow[:, :F_IN].rearrange("p (h c) -> p h c", h=H),
                    in1=exs[:].rearrange("p (h o) -> p h o", o=1).to_broadcast([P, H, C]),
                    op=MULT)
                numer = fpool.tile([P, F_IN], f32, tag="numer")
                nc.vector.tensor_add(out=numer[:], in0=selfsc[:], in1=agg[:, :F_IN])
                dinv = fpool.tile([P, H], f32, tag="dinv")
                nc.vector.tensor_add(out=dinv[:], in0=exs[:], in1=agg[:, F_IN:])
                nc.vector.tensor_scalar_add(out=dinv[:], in0=dinv[:], scalar1=1e-16)
                nc.vector.reciprocal(out=dinv[:], in_=dinv[:])
                o = fpool.tile([P, F_IN], f32, tag="o")
                nc.vector.tensor_tensor(
                    out=o[:].rearrange("p (h c) -> p h c", h=H),
                    in0=numer[:].rearrange("p (h c) -> p h c", h=H),
                    in1=dinv[:].rearrange("p (h o) -> p h o", o=1).to_broadcast([P, H, C]),
                    op=MULT)
                nc.vector.tensor_add(out=o[:], in0=o[:], in1=b1_b[:])
                mmin = fpool.tile([P, F_IN], f32, tag="mmin")
                nc.vector.tensor_scalar_min(out=mmin[:], in0=o[:], scalar1=0.0)
                ex = fpool.tile([P, F_IN], f32, tag="ex")
                nc.scalar.activation(out=ex[:], in_=mmin[:], func=Exp)
                rel = fpool.tile([P, F_IN], f32, tag="rel")
                nc.vector.tensor_scalar_max(out=rel[:], in0=o[:], scalar1=0.0)
                z = fpool.tile([P, F_IN], bf16, tag="z")
                nc.vector.scalar_tensor_tensor(
                    out=z[:], in0=ex[:], scalar=-1.0, in1=rel[:], op0=ADD, op1=ADD)
                nc.sync.dma_start(out=Z[t * P:(t + 1) * P, :], in_=z[:])
                # Phase C for this tile, overlapped with the edge phase
                zt = [cpool2.tile([P, P], bf16, tag=f"zt{k}", name=f"zt{k}")
                      for k in range(2)]
                for k in range(2):
                    nc.sync.dma_start(
                        out=zt[k][:], in_=Z[t * P:(t + 1) * P, k * P:(k + 1) * P],
                        transpose=True)
                ps2 = cpsum.tile([P, D2], f32, tag="cps")
                for k in range(2):
                    nc.tensor.matmul(ps2[:], lhsT=zt[k][:], rhs=rhs2[k][:],
                                     start=(k == 0), stop=(k == 1))
                t2sb = cpool2.tile([P, D2], bf16, tag="t2sb")
                nc.vector.tensor_add(out=t2sb[:], in0=ps2[:], in1=b2p_b[:])
                nc.sync.dma_start(out=T2L[t * P:(t + 1) * P, :], in_=t2sb[:])

            def stage_q(q, t, stpool):
                """Gather tile t's bucket-q T2 rows and stage them to DRAM."""
                kq = Kq[t][q]
                if kq == 0:
                    return
                cstart = int(offs[t]) + cumKq[t][q]
                gstart = int(goffs[t]) + cumKq[t][q]
                ring = stpool.tile([P, KQMAX, D2], bf16, tag="st")
                for j in range(kq):
                    c1 = cstart + j
                    nc.gpsimd.indirect_dma_start(
                        out=ring[:, j, :], out_offset=None,
                        in_=t2f[q].tensor.ap()[:],
                        in_offset=bass.IndirectOffsetOnAxis(
                            ap=srcb2[:, c1:c1 + 1], axis=0),
                    )
                nc.sync.dma_start(out=GE0[:, gstart:gstart + kq, :],
                                  in_=ring[:, :kq, :])

            def phase_b_tile(t, bpool):
                agg = ep_psum.tile([P, F_IN + H], f32, tag="agg")
                selfrow = ep_fpool.tile([P, D1], bf16, tag="selfrow")
                nc.sync.dma_start(out=selfrow[:],
                                  in_=HAUGs[t * P:(t + 1) * P, :])

                def getg1(j0, r, t=t):
                    ci = chunk_of[(t, j0)]
                    hb = bpool.tile([P, R, D1], bf16, tag="hb")
                    xe = bpool.tile([P, 2, R, P], bf16, tag="xe")
                    for k in range(2):
                        nc.sync.dma_start(out=xe[:, k, :r, :],
                                          in_=xetc_d[ci, k, :, :r, :])
                    for ri in range(r):
                        hp = ep_hpsum.tile([P, D1], f32, tag="hp")
                        for k in range(2):
                            nc.tensor.matmul(
                                hp[:], lhsT=xe[:, k, ri, :], rhs=rhs1[k][:],
                                start=(k == 0), stop=(k == 1))
                        nc.scalar.activation(out=hb[:, ri, :], in_=hp[:],
                                             func=Copy)
                    return hb[:]

                _edge_chunks(nc, meta, consts, t=t, selfrow=selfrow,
                             getg=getg1, dglen=F_IN, nheads=H, hw=C, agg=agg)
                flush1(t, agg, selfrow, ep_fpool)

            with contextlib.ExitStack() as pb:
                bpool = pb.enter_context(tc.tile_pool(name="pb", bufs=8))
                stpool = pb.enter_context(tc.tile_pool(name="pst", bufs=6))
                pending = deque()
                for qb in range(nq):
                    for t in range(tqb[qb], tqb[qb + 1]):
                        phase_b_tile(t, bpool)
                        for _ in range(6):
                            if pending:
                                stage_q(*pending.popleft(), stpool)
                    # quarter qb's T2 rows are final -> AllGather this slice
                    nc.gpsimd.collective_compute(
                        "AllGather", mybir.AluOpType.bypass,
                        replica_groups=[list(range(W))],
                        ins=[T2L[tqb[qb] * P:tqb[qb + 1] * P, :]],
                        outs=[t2f[qb].opt()])
                    if qb < nq - 1:
                        pending.extend((qb, t2) for t2 in range(T))
                while pending:
                    stage_q(*pending.popleft(), stpool)

            # ---- Phase D: layer-2 edge phase -> out ----
            def flush2(t, agg, selfrow, fpool):
                es = fpool.tile([P, 1], f32, tag="es2")
                nc.vector.tensor_add(out=es[:], in0=selfrow[:, NCLS:NCLS + 1],
                                     in1=selfrow[:, NCLS + 1:NCLS + 2])
                nc.vector.scalar_tensor_tensor(
                    out=es[:], in0=es[:], scalar=NEG, in1=es[:], op0=MULT, op1=MAX)
                exs = fpool.tile([P, 1], f32, tag="exs2")
                nc.scalar.activation(out=exs[:], in_=es[:], func=Exp)
                selfsc = fpool.tile([P, NCLS], f32, tag="selfsc2")
                nc.vector.tensor_tensor(
                    out=selfsc[:], in0=selfrow[:, :NCLS],
                    in1=exs[:].to_broadcast([P, NCLS]), op=MULT)
                numer = fpool.tile([P, NCLS], f32, tag="numer2")
                nc.vector.tensor_add(out=numer[:], in0=selfsc[:], in1=agg[:, :NCLS])
                dinv = fpool.tile([P, 1], f32, tag="dinv2")
                nc.vector.tensor_add(out=dinv[:], in0=exs[:], in1=agg[:, NCLS:])
                nc.vector.tensor_scalar_add(out=dinv[:], in0=dinv[:], scalar1=1e-16)
                nc.vector.reciprocal(out=dinv[:], in_=dinv[:])
                o = fpool.tile([P, NCLS], f32, tag="o2")
                nc.vector.tensor_tensor(
                    out=o[:], in0=numer[:], in1=dinv[:].to_broadcast([P, NCLS]), op=MULT)
                nc.sync.dma_start(out=out_d[t * P:(t + 1) * P, :], in_=o[:])

            with contextlib.ExitStack() as pd:
                dpool = pd.enter_context(tc.tile_pool(name="pd", bufs=8))
                t2f_last = t2f[nq - 1].tensor.ap()
                for t in range(T):
                    agg = ep_psum.tile([P, NCLS + 1], f32, tag="agg")
                    base = int(offs[t])
                    g0 = int(goffs[t])
                    ks = KS[t]
                    selfrow = ep_fpool.tile([P, D2], bf16, tag="selfrow2")
                    nc.sync.dma_start(out=selfrow[:],
                                      in_=T2L[t * P:(t + 1) * P, :])

                    def getg2(j0, r, base=base, g0=g0, ks=ks):
                        g = dpool.tile([P, R, D2], bf16, tag="g2")
                        nb0 = max(0, min(ks - j0, r))
                        if nb0 > 0:         # staged bucket rows
                            nc.sync.dma_start(
                                out=g[:, :nb0, :],
                                in_=GE0[:, g0 + j0:g0 + j0 + nb0, :])
                        for ri in range(nb0, r):   # final-bucket gathers
                            c1 = base + j0 + ri
                            nc.gpsimd.indirect_dma_start(
                                out=g[:, ri, :], out_offset=None, in_=t2f_last[:],
                                in_offset=bass.IndirectOffsetOnAxis(
                                    ap=srcb2[:, c1:c1 + 1], axis=0),
                            )
                        return g[:]

                    _edge_chunks(nc, meta, consts, t=t, selfrow=selfrow,
                                 getg=getg2, dglen=NCLS, nheads=1, hw=NCLS,
                                 agg=agg)
                    flush2(t, agg, selfrow, ep_fpool)

    nc.compile()
    return nc


def kernel(**inputs):
    x = np.asarray(inputs["x"], np.float32)
    edge_index = np.asarray(inputs["edge_index"])
    meta, arrays = _host_prep(x, edge_index)

    key = (meta["N"], meta["E"], meta["K"], meta["KS"])
    if key not in _CACHE:
        _CACHE[key] = _build_program(meta)
    nc = _CACHE[key]

    common = {
        "W1": np.asarray(inputs["W1"], np.float32),
        "asrc": np.asarray(inputs["att_src1"], np.float32).reshape(1, -1),
        "adstv": np.asarray(inputs["att_dst1"], np.float32).reshape(1, -1),
        "b1": np.asarray(inputs["bias1"], np.float32).reshape(1, -1),
        "W2": np.asarray(inputs["W2"], np.float32),
        "a2s": np.asarray(inputs["att_src2"], np.float32).reshape(1, -1),
        "a2d": np.asarray(inputs["att_dst2"], np.float32).reshape(1, -1),
        "b2": np.asarray(inputs["bias2"], np.float32).reshape(1, -1),
    }
    in_maps = []
    for c in range(W):
        m = dict(common)
        m["XETC"] = arrays["build_xetc"](c)
        m["xTs"] = arrays["xts"][c]
        m["srcg2"] = arrays["t2row"][c]
        m["sstc"] = _make_sstc(arrays["dstl_arr"][c], meta)
        in_maps.append(m)

    res = run_bass_kernel_spmd(nc, in_maps, core_ids=list(range(W)), trace=TRACE)
    kernel.last_results = res

    N = meta["N"]
    out = np.concatenate([res.results[c]["out"] for c in range(W)], axis=0)
    return np.ascontiguousarray(out[:N])


# revision 26
# speedup vs baseline: 1.0238x; 1.0138x over previous
"""Two-layer GAT (PyG GATConv semantics) on 8 TRN2 NeuronCores.

Strategy (edge/graph parallel, v3):
  - Host (index manipulation only): sort non-self-loop edges by dst, shard
    dst nodes contiguously across the 8 cores, pad each dst tile's edge
    list to a multiple of 128, and precompute each 128-edge tile's 0/1
    incidence matrices S (dst x edge) and S^T as bf16 (pure index data).
    The host also pre-permutes x rows into per-edge-tile transposed blocks
    XET[c] = x[src[tile c]].T (a gather/reshape of the input - no
    arithmetic) so layer 1 needs NO device-side indirect DMA.  sst/XET are
    stored chunk-blocked so each load is ~128 descriptors of 1-2KB.
  - Phase A (per core, own dst shard): HAUGs[v] = [x@W1 | a_src | a_dst]
    (272 cols, bf16); bias b1 is applied post-softmax in the flush.
  - Phase B (edge phase, layer 1): per 128-edge tile, he = XET^T @ W1aug
    on the tensor engine; alpha_dst per edge via a small matmul
    S^T @ a_dst_tile; exp(leakyrelu(asrc+adst)); one incidence matmul
    accumulates [sum exp*h | sum exp] in PSUM.  Self-loops are added
    densely in the flush.  Normalize, +b1, ELU, store z; layer-2 rows
    T2[v] = [z@W2 + b2 | a2_src | a2_dst] computed inline (Phase C).
  - The T2 AllGather is split in two halves; layer-2 edges are bucketed by
    src half.  Bucket-0 T2 rows are indirect-gathered and staged to DRAM
    concurrently with the second half of Phase B (GpSimd is otherwise
    idle there), so only bucket-1 gathers remain serialized in Phase D.
  - Phase D: layer-2 edge phase (heads=1) -> output shard.

All floating-point math happens on device; the host only reorders
indices/rows and pads/reshapes layouts.
"""

import contextlib

import numpy as np

import concourse.bass as bass
import concourse.bacc as bacc
import concourse.mybir as mybir
import concourse.tile as tile
from concourse.bass_utils import run_bass_kernel_spmd

# ---- fixed problem hyperparameters (from the nn.Module) ----
F_IN = 256
H = 8
C = 32
NCLS = 40
NEG = 0.2

W = 8               # cores
P = 128             # partitions
D1 = F_IN + 2 * H   # HAUG row: [h (256) | a_src (8) | a_dst (8)] = 272
D2 = NCLS + 8       # T2 row: [h2+b2 (40) | a2_src (1) | a2_dst x7 (41:48)]
R = 4               # edge tiles batched per compute group

f32 = mybir.dt.float32
bf16 = mybir.dt.bfloat16
i32 = mybir.dt.int32
BF_NP = mybir.dt.np(bf16)

Exp = mybir.ActivationFunctionType.Exp
Copy = mybir.ActivationFunctionType.Copy
ADD = mybir.AluOpType.add
MULT = mybir.AluOpType.mult
MAX = mybir.AluOpType.max

TRACE = False       # set by test harness for profiling runs
_CACHE = {}


def _host_prep(x, edge_index):
    """Index-only preprocessing. Returns (meta, per-core arrays)."""
    N = x.shape[0]
    E = edge_index.shape[1]
    src_a = np.asarray(edge_index[0], np.int64)
    dst_a = np.asarray(edge_index[1], np.int64)
    order = np.argsort(dst_a, kind="stable")
    src_s = src_a[order].astype(np.int64)
    dst_s = dst_a[order].astype(np.int64)

    nt_real = -(-N // P)
    T = -(-nt_real // W)
    SH = T * P
    SH2 = SH // 2
    NPAD = W * SH

    # bucket edges by (core, dst tile, src half) for the split AllGather
    gt = dst_s // P                      # global dst tile
    half = (src_s % SH) // SH2           # 0 / 1
    key = gt * 2 + half
    cnt = np.bincount(key, minlength=W * T * 2).reshape(W, T, 2)
    K01 = -(-cnt.max(axis=0) // P)       # [T, 2] tiles per bucket
    K01[:, 0] = np.maximum(K01[:, 0], 1)
    K0 = K01[:, 0]
    K1 = K01[:, 1]
    K = K0 + K1
    offs = np.zeros(T + 1, np.int64)
    offs[1:] = np.cumsum(K)
    CT = int(offs[-1])

    # rank of each edge within its (core, tile, half) bucket
    sort2 = np.argsort(key, kind="stable")
    src_s = src_s[sort2]
    dst_s = dst_s[sort2]
    gt = gt[sort2]
    half = half[sort2]
    key = key[sort2]
    start_k = np.zeros(W * T * 2 + 1, np.int64)
    start_k[1:] = np.cumsum(cnt.reshape(-1))
    q = np.arange(len(dst_s)) - start_k[key]
    c_all = gt // T
    t_all = gt % T
    col = offs[t_all] + np.where(half == 0, 0, K0[t_all]) + q // P
    p_all = q % P

    src_arr = np.zeros((W, P, CT), np.int64)
    dstl_arr = np.full((W, P, CT), -1, np.int16)
    src_arr[c_all, p_all, col] = src_s
    dstl_arr[c_all, p_all, col] = (dst_s % P).astype(np.int16)

    # layer-2 gather rows in the split-T2F layout:
    # node v = s*SH + l -> T2F[l // SH2] row s*SH2 + (l % SH2)
    t2row = (src_arr // SH) * SH2 + (src_arr % SH) % SH2
    t2row = t2row.astype(np.int32)
    src_arr = src_arr.astype(np.int32)

    # chunk table: one entry per (t, j0) compute group
    chunks = []                          # (t, base, j0, r)
    chunk_of = {}
    for t in range(T):
        for j0 in range(0, int(K[t]), R):
            r = min(R, int(K[t]) - j0)
            chunk_of[(t, j0)] = len(chunks)
            chunks.append((t, int(offs[t]), j0, r))
    NCHT = len(chunks)

    offs0 = np.zeros(T + 1, np.int64)
    offs0[1:] = np.cumsum(K0)
    CT0 = int(offs0[-1])

    meta = dict(N=N, E=E, T=T, SH=SH, SH2=SH2, NPAD=NPAD,
                K=tuple(int(k) for k in K), K0=tuple(int(k) for k in K0),
                offs=offs, offs0=offs0, CT=CT, CT0=CT0, NCHT=NCHT,
                chunks=tuple(chunks), chunk_of=chunk_of)

    xb = np.zeros((NPAD, F_IN), BF_NP)
    xb[:N] = np.asarray(x, np.float32).astype(BF_NP)

    def build_xetc(c):
        # [NCHT, P(feat-half), 2*R*P] chunk-blocked transposed x rows,
        # flattened per partition so each chunk loads in ONE 128-desc DMA
        out = np.zeros((NCHT, P, 2, R, P), BF_NP)
        for ci, (t, base, j0, r) in enumerate(chunks):
            cols = src_arr[c][:, base + j0:base + j0 + r]     # [P(edge), r]
            blk = xb[cols]                                    # [Pe, r, 256]
            out[ci, :, :, :r, :] = (blk.transpose(2, 1, 0)    # [256f, r, Pe]
                                    .reshape(2, P, r, P)      # [k, f, r, e]
                                    .transpose(1, 0, 2, 3))   # [f, k, r, e]
        return out.reshape(NCHT, P, 2 * R * P)

    xts = [np.ascontiguousarray(xb[c * SH:(c + 1) * SH].T) for c in range(W)]

    arrays = dict(src_arr=src_arr, dstl_arr=dstl_arr, t2row=t2row,
                  build_xetc=build_xetc, xts=xts)
    return meta, arrays


def _make_sstc(dstl_core, meta):
    """[P, CT] int16 dst-local (-1 pad) -> [NCHT, P, R, 256] bf16 [S | S^T]."""
    chunks = meta["chunks"]
    NCHT = meta["NCHT"]
    iota = np.arange(P, dtype=np.int16)
    out = np.zeros((NCHT, P, R, 2 * P), BF_NP)
    for ci, (t, base, j0, r) in enumerate(chunks):
        d = dstl_core[:, base + j0:base + j0 + r].T          # [r, 128e]
        S = (d[:, None, :] == iota[None, :, None])           # [r, d, e]
        out[ci, :, :r, :P] = S.transpose(1, 0, 2)
        out[ci, :, :r, P:] = S.transpose(2, 0, 1)
    return out


def _edge_chunks(nc, meta, consts, *, t, selfrow, getg, dglen, nheads, hw,
                 agg):
    """Shared per-chunk compute: attention weights + incidence aggregation."""
    K = meta["K"]
    chunk_of = meta["chunk_of"]
    sstc_d = consts["sstc_d"]
    pool = consts["pool"]
    spool = consts["spool"]
    apsum = consts["apsum"]
    DA = dglen + nheads
    nj = K[t]

    for j0 in range(0, nj, R):
        r = min(R, nj - j0)
        ci = chunk_of[(t, j0)]
        g = getg(j0, r)
        sst = spool.tile([P, R, 2 * P], bf16, tag="sst")
        nc.scalar.dma_start(out=sst[:, :r, :], in_=sstc_d[ci, :, :r, :])
        ade = apsum.tile([P, R, nheads], f32, tag="ade")
        for ri in range(r):
            nc.tensor.matmul(
                ade[:, ri, :], lhsT=sst[:, ri, :P],
                rhs=selfrow[:, dglen + nheads:dglen + 2 * nheads],
                start=True, stop=True)
        s = pool.tile([P, R, nheads], f32, tag="s")
        nc.vector.tensor_add(out=s[:, :r], in0=g[:, :r, dglen:dglen + nheads],
                             in1=ade[:, :r])
        e = pool.tile([P, R, nheads], f32, tag="e")
        nc.vector.scalar_tensor_tensor(
            out=e[:, :r], in0=s[:, :r], scalar=NEG, in1=s[:, :r],
            op0=MULT, op1=MAX)
        rhs = pool.tile([P, R, DA], bf16, tag="rhs")
        nc.scalar.activation(out=rhs[:, :r, dglen:], in_=e[:, :r], func=Exp)
        nc.vector.tensor_tensor(
            out=rhs[:, :r, :dglen].rearrange("p r (h c) -> p r h c", h=nheads),
            in0=g[:, :r, :dglen].rearrange("p r (h c) -> p r h c", h=nheads),
            in1=rhs[:, :r, dglen:].rearrange("p r (h o) -> p r h o", o=1)
                .to_broadcast([P, r, nheads, hw]),
            op=MULT)
        for ri in range(r):
            nc.tensor.matmul(
                agg[:], lhsT=sst[:, ri, P:], rhs=rhs[:, ri, :],
                start=(j0 == 0 and ri == 0), stop=(j0 + ri == nj - 1))


def _build_program(meta):
    T, SH, SH2, NPAD = meta["T"], meta["SH"], meta["SH2"], meta["NPAD"]
    CT, CT0, NCHT = meta["CT"], meta["CT0"], meta["NCHT"]
    K, K0, offs, offs0 = meta["K"], meta["K0"], meta["offs"], meta["offs0"]
    chunk_of = meta["chunk_of"]
    K0MAX = max(max(K0), 1)

    nc = bacc.Bacc("TRN2", target_bir_lowering=False, debug=False, num_devices=W)

    xetc_d = nc.dram_tensor("XETC", [NCHT, P, 2 * R * P], bf16, kind="ExternalInput")
    xts_d = nc.dram_tensor("xTs", [F_IN, SH], bf16, kind="ExternalInput")
    w1_d = nc.dram_tensor("W1", [F_IN, F_IN], f32, kind="ExternalInput")
    asrc_d = nc.dram_tensor("asrc", [1, F_IN], f32, kind="ExternalInput")
    adstv_d = nc.dram_tensor("adstv", [1, F_IN], f32, kind="ExternalInput")
    b1_d = nc.dram_tensor("b1", [1, F_IN], f32, kind="ExternalInput")
    w2_d = nc.dram_tensor("W2", [F_IN, NCLS], f32, kind="ExternalInput")
    a2s_d = nc.dram_tensor("a2s", [1, NCLS], f32, kind="ExternalInput")
    a2d_d = nc.dram_tensor("a2d", [1, NCLS], f32, kind="ExternalInput")
    b2_d = nc.dram_tensor("b2", [1, NCLS], f32, kind="ExternalInput")
    srcg2_d = nc.dram_tensor("srcg2", [P, CT], i32, kind="ExternalInput")
    sstc_d = nc.dram_tensor("sstc", [NCHT, P, R, 2 * P], bf16, kind="ExternalInput")
    out_d = nc.dram_tensor("out", [SH, NCLS], f32, kind="ExternalOutput")

    HAUGs = nc.dram_tensor("HAUGs", [SH, D1], bf16)
    Z = nc.dram_tensor("Z", [SH, F_IN], bf16)
    GE0 = nc.dram_tensor("GE0", [P, max(CT0, 1), D2], bf16)

    with tile.TileContext(nc) as tc:
        with contextlib.ExitStack() as top:
            cpool = top.enter_context(tc.tile_pool(name="const", bufs=1))
            dram = top.enter_context(tc.tile_pool(name="dram", bufs=1, space="DRAM"))

            srcb2 = cpool.tile([P, CT], i32)
            nc.sync.dma_start(out=srcb2[:], in_=srcg2_d[:])

            rhs1 = [cpool.tile([P, D1], bf16, name=f"rhs1_{k}") for k in range(2)]
            rhs2 = [cpool.tile([P, D2], bf16, name=f"rhs2_{k}") for k in range(2)]
            b1_b = cpool.tile([P, F_IN], f32)
            b2p_b = cpool.tile([P, D2], f32)

            # ---- setup: broadcast rows + fold attention vectors into rhs ----
            with contextlib.ExitStack() as su:
                spool = su.enter_context(tc.tile_pool(name="setup", bufs=1))
                spsum = su.enter_context(tc.tile_pool(name="setup_ps", bufs=1, space="PSUM"))
                ones = spool.tile([1, P], f32)
                nc.vector.memset(ones[:], 1.0)

                def bcast(dram_ap, width, out_ap):
                    ps = spsum.tile([P, width], f32, tag="bps")
                    row = spool.tile([1, width], f32, tag="brow")
                    nc.sync.dma_start(out=row[:], in_=dram_ap)
                    nc.tensor.matmul(ps[:], lhsT=ones[:], rhs=row[:], start=True, stop=True)
                    nc.vector.tensor_copy(out=out_ap, in_=ps[:])

                asrc_b = spool.tile([P, F_IN], f32)
                bcast(asrc_d[:], F_IN, asrc_b[:])
                adst_b = spool.tile([P, F_IN], f32)
                bcast(adstv_d[:], F_IN, adst_b[:])
                bcast(b1_d[:], F_IN, b1_b[:])
                a2s_b = spool.tile([P, NCLS], f32)
                bcast(a2s_d[:], NCLS, a2s_b[:])
                a2d_b = spool.tile([P, NCLS], f32)
                bcast(a2d_d[:], NCLS, a2d_b[:])
                nc.vector.memset(b2p_b[:], 0.0)
                bcast(b2_d[:], NCLS, b2p_b[:, :NCLS])

                for k in range(2):
                    w1sb = spool.tile([P, F_IN], f32, tag="w1sb")
                    nc.sync.dma_start(out=w1sb[:], in_=w1_d[k * P:(k + 1) * P, :])
                    nc.vector.tensor_copy(out=rhs1[k][:, :F_IN], in_=w1sb[:])
                    for vec_b, col in ((asrc_b, F_IN), (adst_b, F_IN + H)):
                        tmp = spool.tile([P, F_IN], f32, tag="tmp")
                        nc.vector.tensor_mul(out=tmp[:], in0=w1sb[:], in1=vec_b[:])
                        vred = spool.tile([P, H], f32, tag="vred")
                        nc.vector.tensor_reduce(
                            out=vred[:], in_=tmp[:].rearrange("p (h c) -> p h c", h=H),
                            axis=mybir.AxisListType.X, op=ADD)
                        nc.vector.tensor_copy(out=rhs1[k][:, col:col + H], in_=vred[:])

                    w2sb = spool.tile([P, NCLS], f32, tag="w2sb")
                    nc.sync.dma_start(out=w2sb[:], in_=w2_d[k * P:(k + 1) * P, :])
                    nc.vector.tensor_copy(out=rhs2[k][:, :NCLS], in_=w2sb[:])
                    for vec_b, cs in ((a2s_b, slice(NCLS, NCLS + 1)),
                                      (a2d_b, slice(NCLS + 1, D2))):
                        tmp2 = spool.tile([P, NCLS], f32, tag="tmp2")
                        nc.vector.tensor_mul(out=tmp2[:], in0=w2sb[:], in1=vec_b[:])
                        vred2 = spool.tile([P, 1], f32, tag="vred2")
                        nc.vector.tensor_reduce(
                            out=vred2[:], in_=tmp2[:].rearrange("p (o c) -> p o c", o=1),
                            axis=mybir.AxisListType.X, op=ADD)
                        n_rep = cs.stop - cs.start
                        nc.vector.tensor_copy(
                            out=rhs2[k][:, cs], in_=vred2[:].to_broadcast([P, n_rep]))

            # ---- Phase A: HAUGs (bias-free) for own dst shard ----
            with contextlib.ExitStack() as pa:
                apool = pa.enter_context(tc.tile_pool(name="pa", bufs=4))
                apsum = pa.enter_context(tc.tile_pool(name="pa_ps", bufs=2, space="PSUM"))
                BS = min(14, T)
                for b0 in range(0, T, BS):
                    nb = min(BS, T - b0)
                    xt = [apool.tile([P, BS * P], bf16, tag=f"xt{k}", name=f"xt{k}")
                          for k in range(2)]
                    for k in range(2):
                        nc.sync.dma_start(
                            out=xt[k][:, :nb * P],
                            in_=xts_d[k * P:(k + 1) * P, b0 * P:(b0 + nb) * P])
                    hsb = apool.tile([P, BS, D1], bf16, tag="hsb")
                    for nt in range(nb):
                        ps = apsum.tile([P, D1], f32, tag="aps")
                        for k in range(2):
                            nc.tensor.matmul(
                                ps[:], lhsT=xt[k][:, nt * P:(nt + 1) * P], rhs=rhs1[k][:],
                                start=(k == 0), stop=(k == 1))
                        nc.scalar.activation(out=hsb[:, nt, :], in_=ps[:],
                                             func=Copy)
                    row0 = b0 * P
                    nc.scalar.dma_start(
                        out=HAUGs[row0:row0 + nb * P, :].rearrange("(a p) d -> p a d", p=P),
                        in_=hsb[:, :nb, :])

            cpool2 = top.enter_context(tc.tile_pool(name="pc", bufs=3))
            cpsum = top.enter_context(tc.tile_pool(name="pc_ps", bufs=1, space="PSUM"))
            T2L = dram.tile([SH, D2], bf16, name="T2L")
            ep_pool = top.enter_context(tc.tile_pool(name="ep", bufs=8))
            ep_spool = top.enter_context(tc.tile_pool(name="ep_s", bufs=6))
            ep_fpool = top.enter_context(tc.tile_pool(name="ep_f", bufs=3))
            ep_psum = top.enter_context(tc.tile_pool(name="ep_p", bufs=2, space="PSUM"))
            ep_apsum = top.enter_context(tc.tile_pool(name="ep_a", bufs=2, space="PSUM"))
            ep_hpsum = top.enter_context(tc.tile_pool(name="ep_h", bufs=3, space="PSUM"))
            consts = dict(sstc_d=sstc_d, pool=ep_pool, spool=ep_spool,
                          fpool=ep_fpool, psum=ep_psum, apsum=ep_apsum)

            # T2 AllGather halves (layer-2 gather tables)
            T2F0 = dram.tile([W * SH2, D2], bf16, name="T2F0", addr_space="Shared")
            T2F1 = dram.tile([W * SH2, D2], bf16, name="T2F1", addr_space="Shared")

            # ---- Phase B: layer-1 edge phase (dense) -> Z, T2L ----
            def flush1(t, agg, selfrow, fpool):
                es = fpool.tile([P, H], f32, tag="es")
                nc.vector.tensor_add(out=es[:], in0=selfrow[:, F_IN:F_IN + H],
                                     in1=selfrow[:, F_IN + H:])
                nc.vector.scalar_tensor_tensor(
                    out=es[:], in0=es[:], scalar=NEG, in1=es[:], op0=MULT, op1=MAX)
                exs = fpool.tile([P, H], f32, tag="exs")
                nc.scalar.activation(out=exs[:], in_=es[:], func=Exp)
                selfsc = fpool.tile([P, F_IN], f32, tag="selfsc")
                nc.vector.tensor_tensor(
                    out=selfsc[:].rearrange("p (h c) -> p h c", h=H),
                    in0=selfrow[:, :F_IN].rearrange("p (h c) -> p h c", h=H),
                    in1=exs[:].rearrange("p (h o) -> p h o", o=1).to_broadcast([P, H, C]),
                    op=MULT)
                numer = fpool.tile([P, F_IN], f32, tag="numer")
                nc.vector.tensor_add(out=numer[:], in0=selfsc[:], in1=agg[:, :F_IN])
                dinv = fpool.tile([P, H], f32, tag="dinv")
                nc.vector.tensor_add(out=dinv[:], in0=exs[:], in1=agg[:, F_IN:])
                nc.vector.tensor_scalar_add(out=dinv[:], in0=dinv[:], scalar1=1e-16)
                nc.vector.reciprocal(out=dinv[:], in_=dinv[:])
                o = fpool.tile([P, F_IN], f32, tag="o")
                nc.vector.tensor_tensor(
                    out=o[:].rearrange("p (h c) -> p h c", h=H),
                    in0=numer[:].rearrange("p (h c) -> p h c", h=H),
                    in1=dinv[:].rearrange("p (h o) -> p h o", o=1).to_broadcast([P, H, C]),
                    op=MULT)
                nc.vector.tensor_add(out=o[:], in0=o[:], in1=b1_b[:])
                mmin = fpool.tile([P, F_IN], f32, tag="mmin")
                nc.vector.tensor_scalar_min(out=mmin[:], in0=o[:], scalar1=0.0)
                ex = fpool.tile([P, F_IN], f32, tag="ex")
                nc.scalar.activation(out=ex[:], in_=mmin[:], func=Exp)
                rel = fpool.tile([P, F_IN], f32, tag="rel")
                nc.vector.tensor_scalar_max(out=rel[:], in0=o[:], scalar1=0.0)
                z = fpool.tile([P, F_IN], bf16, tag="z")
                nc.vector.scalar_tensor_tensor(
                    out=z[:], in0=ex[:], scalar=-1.0, in1=rel[:], op0=ADD, op1=ADD)
                nc.sync.dma_start(out=Z[t * P:(t + 1) * P, :], in_=z[:])
                # Phase C for this tile, overlapped with the edge phase
                zt = [cpool2.tile([P, P], bf16, tag=f"zt{k}", name=f"zt{k}")
                      for k in range(2)]
                for k in range(2):
                    nc.sync.dma_start(
                        out=zt[k][:], in_=Z[t * P:(t + 1) * P, k * P:(k + 1) * P],
                        transpose=True)
                ps2 = cpsum.tile([P, D2], f32, tag="cps")
                for k in range(2):
                    nc.tensor.matmul(ps2[:], lhsT=zt[k][:], rhs=rhs2[k][:],
                                     start=(k == 0), stop=(k == 1))
                t2sb = cpool2.tile([P, D2], bf16, tag="t2sb")
                nc.vector.tensor_add(out=t2sb[:], in0=ps2[:], in1=b2p_b[:])
                nc.sync.dma_start(out=T2L[t * P:(t + 1) * P, :], in_=t2sb[:])

            t2f0_ap = T2F0.tensor.ap()
            t2f1_ap = T2F1.tensor.ap()

            def stage0(t, stpool):
                """Gather this tile's bucket-0 T2 rows and stage them to DRAM
                (runs on gpsimd+sync, concurrent with phase-B compute)."""
                k0 = K0[t]
                if k0 == 0:
                    return
                b0 = int(offs0[t])
                base = int(offs[t])
                ring = stpool.tile([P, K0MAX, D2], bf16, tag="st")
                for j in range(k0):
                    c1 = base + j
                    nc.gpsimd.indirect_dma_start(
                        out=ring[:, j, :], out_offset=None, in_=t2f0_ap[:],
                        in_offset=bass.IndirectOffsetOnAxis(
                            ap=srcb2[:, c1:c1 + 1], axis=0),
                    )
                nc.scalar.dma_start(out=GE0[:, b0:b0 + k0, :], in_=ring[:, :k0, :])

            def phase_b_tile(t, bpool):
                agg = ep_psum.tile([P, F_IN + H], f32, tag="agg")
                selfrow = ep_fpool.tile([P, D1], bf16, tag="selfrow")
                nc.sync.dma_start(out=selfrow[:],
                                  in_=HAUGs[t * P:(t + 1) * P, :])

                def getg1(j0, r, t=t):
                    ci = chunk_of[(t, j0)]
                    hb = bpool.tile([P, R, D1], bf16, tag="hb")
                    xe = bpool.tile([P, 2, R, P], bf16, tag="xe")
                    nc.sync.dma_start(
                        out=xe[:].rearrange("p k r e -> p (k r e)"),
                        in_=xetc_d[ci, :, :])
                    for ri in range(r):
                        hp = ep_hpsum.tile([P, D1], f32, tag="hp")
                        for k in range(2):
                            nc.tensor.matmul(
                                hp[:], lhsT=xe[:, k, ri, :], rhs=rhs1[k][:],
                                start=(k == 0), stop=(k == 1))
                        if ri % 2 == 0:
                            nc.scalar.activation(out=hb[:, ri, :], in_=hp[:],
                                                 func=Copy)
                        else:
                            nc.vector.tensor_copy(out=hb[:, ri, :], in_=hp[:])
                    return hb[:]

                _edge_chunks(nc, meta, consts, t=t, selfrow=selfrow,
                             getg=getg1, dglen=F_IN, nheads=H, hw=C, agg=agg)
                flush1(t, agg, selfrow, ep_fpool)

            TH = (T + 1) // 2
            with contextlib.ExitStack() as pb:
                bpool = pb.enter_context(tc.tile_pool(name="pb", bufs=8))
                stpool = pb.enter_context(tc.tile_pool(name="pst", bufs=4))
                for t in range(TH):
                    phase_b_tile(t, bpool)
                # first-half T2 rows are final -> AllGather half 0
                nc.gpsimd.collective_compute(
                    "AllGather", mybir.AluOpType.bypass,
                    replica_groups=[list(range(W))],
                    ins=[T2L[0:SH2, :]], outs=[T2F0.opt()])
                # second half of phase B, with bucket-0 layer-2 gathers
                # (2 staging tiles per compute tile) interleaved
                st_t = 0
                for t in range(TH, T):
                    phase_b_tile(t, bpool)
                    for _ in range(2):
                        if st_t < T:
                            stage0(st_t, stpool)
                            st_t += 1
                while st_t < T:
                    stage0(st_t, stpool)
                    st_t += 1
                nc.gpsimd.collective_compute(
                    "AllGather", mybir.AluOpType.bypass,
                    replica_groups=[list(range(W))],
                    ins=[T2L[SH2:SH, :]], outs=[T2F1.opt()])

            # ---- Phase D: layer-2 edge phase -> out ----
            def flush2(t, agg, selfrow, fpool):
                es = fpool.tile([P, 1], f32, tag="es2")
                nc.vector.tensor_add(out=es[:], in0=selfrow[:, NCLS:NCLS + 1],
                                     in1=selfrow[:, NCLS + 1:NCLS + 2])
                nc.vector.scalar_tensor_tensor(
                    out=es[:], in0=es[:], scalar=NEG, in1=es[:], op0=MULT, op1=MAX)
                exs = fpool.tile([P, 1], f32, tag="exs2")
                nc.scalar.activation(out=exs[:], in_=es[:], func=Exp)
                selfsc = fpool.tile([P, NCLS], f32, tag="selfsc2")
                nc.vector.tensor_tensor(
                    out=selfsc[:], in0=selfrow[:, :NCLS],
                    in1=exs[:].to_broadcast([P, NCLS]), op=MULT)
                numer = fpool.tile([P, NCLS], f32, tag="numer2")
                nc.vector.tensor_add(out=numer[:], in0=selfsc[:], in1=agg[:, :NCLS])
                dinv = fpool.tile([P, 1], f32, tag="dinv2")
                nc.vector.tensor_add(out=dinv[:], in0=exs[:], in1=agg[:, NCLS:])
                nc.vector.tensor_scalar_add(out=dinv[:], in0=dinv[:], scalar1=1e-16)
                nc.vector.reciprocal(out=dinv[:], in_=dinv[:])
                o = fpool.tile([P, NCLS], f32, tag="o2")
                nc.vector.tensor_tensor(
                    out=o[:], in0=numer[:], in1=dinv[:].to_broadcast([P, NCLS]), op=MULT)
                nc.sync.dma_start(out=out_d[t * P:(t + 1) * P, :], in_=o[:])

            with contextlib.ExitStack() as pd:
                dpool = pd.enter_context(tc.tile_pool(name="pd", bufs=8))
                for t in range(T):
                    agg = ep_psum.tile([P, NCLS + 1], f32, tag="agg")
                    base = int(offs[t])
                    b0 = int(offs0[t])
                    k0 = K0[t]
                    selfrow = ep_fpool.tile([P, D2], bf16, tag="selfrow2")
                    nc.sync.dma_start(out=selfrow[:],
                                      in_=T2L[t * P:(t + 1) * P, :])

                    def getg2(j0, r, base=base, b0=b0, k0=k0):
                        g = dpool.tile([P, R, D2], bf16, tag="g2")
                        nb0 = max(0, min(k0 - j0, r))
                        if nb0 > 0:         # staged bucket-0 rows
                            nc.sync.dma_start(
                                out=g[:, :nb0, :],
                                in_=GE0[:, b0 + j0:b0 + j0 + nb0, :])
                        for ri in range(nb0, r):   # bucket-1 gathers
                            c1 = base + j0 + ri
                            nc.gpsimd.indirect_dma_start(
                                out=g[:, ri, :], out_offset=None, in_=t2f1_ap[:],
                                in_offset=bass.IndirectOffsetOnAxis(
                                    ap=srcb2[:, c1:c1 + 1], axis=0),
                            )
                        return g[:]

                    _edge_chunks(nc, meta, consts, t=t, selfrow=selfrow,
                                 getg=getg2, dglen=NCLS, nheads=1, hw=NCLS,
                                 agg=agg)
                    flush2(t, agg, selfrow, ep_fpool)

    nc.compile()
    return nc


def kernel(**inputs):
    x = np.asarray(inputs["x"], np.float32)
    edge_index = np.asarray(inputs["edge_index"])
    meta, arrays = _host_prep(x, edge_index)

    key = (meta["N"], meta["E"], meta["K"], meta["K0"])
    if key not in _CACHE:
        _CACHE[key] = _build_program(meta)
    nc = _CACHE[key]

    common = {
        "W1": np.asarray(inputs["W1"], np.float32),
        "asrc": np.asarray(inputs["att_src1"], np.float32).reshape(1, -1),
        "adstv": np.asarray(inputs["att_dst1"], np.float32).reshape(1, -1),
        "b1": np.asarray(inputs["bias1"], np.float32).reshape(1, -1),
        "W2": np.asarray(inputs["W2"], np.float32),
        "a2s": np.asarray(inputs["att_src2"], np.float32).reshape(1, -1),
        "a2d": np.asarray(inputs["att_dst2"], np.float32).reshape(1, -1),
        "b2": np.asarray(inputs["bias2"], np.float32).reshape(1, -1),
    }
    in_maps = []
    for c in range(W):
        m = dict(common)
        m["XETC"] = arrays["build_xetc"](c)
        m["xTs"] = arrays["xts"][c]
        m["srcg2"] = arrays["t2row"][c]
        m["sstc"] = _make_sstc(arrays["dstl_arr"][c], meta)
        in_maps.append(m)

    res = run_bass_kernel_spmd(nc, in_maps, core_ids=list(range(W)), trace=TRACE)
    kernel.last_results = res

    N = meta["N"]
    out = np.concatenate([res.results[c]["out"] for c in range(W)], axis=0)
    return np.ascontiguousarray(out[:N])


# revision 27
# speedup vs baseline: 1.0754x; 1.0504x over previous
"""Two-layer GAT (PyG GATConv semantics) on 8 TRN2 NeuronCores.

Strategy (edge/graph parallel, v3):
  - Host (index manipulation only): sort non-self-loop edges by dst, shard
    dst nodes contiguously across the 8 cores, pad each dst tile's edge
    list to a multiple of 128, and precompute each 128-edge tile's 0/1
    incidence matrices S (dst x edge) and S^T as bf16 (pure index data).
    The host also pre-permutes x rows into per-edge-tile transposed blocks
    XET[c] = x[src[tile c]].T (a gather/reshape of the input - no
    arithmetic) so layer 1 needs NO device-side indirect DMA.  sst/XET are
    stored chunk-blocked so each load is ~128 descriptors of 1-2KB.
  - Phase A (per core, own dst shard): HAUGs[v] = [x@W1 | a_src | a_dst]
    (272 cols, bf16); bias b1 is applied post-softmax in the flush.
  - Phase B (edge phase, layer 1): per 128-edge tile, he = XET^T @ W1aug
    on the tensor engine; alpha_dst per edge via a small matmul
    S^T @ a_dst_tile; exp(leakyrelu(asrc+adst)); one incidence matmul
    accumulates [sum exp*h | sum exp] in PSUM.  Self-loops are added
    densely in the flush.  Normalize, +b1, ELU, store z; layer-2 rows
    T2[v] = [z@W2 + b2 | a2_src | a2_dst] computed inline (Phase C).
  - The T2 AllGather is split in two halves; layer-2 edges are bucketed by
    src half.  Bucket-0 T2 rows are indirect-gathered and staged to DRAM
    concurrently with the second half of Phase B (GpSimd is otherwise
    idle there), so only bucket-1 gathers remain serialized in Phase D.
  - Phase D: layer-2 edge phase (heads=1) -> output shard.

All floating-point math happens on device; the host only reorders
indices/rows and pads/reshapes layouts.
"""

import contextlib

import numpy as np

import concourse.bass as bass
import concourse.bacc as bacc
import concourse.mybir as mybir
import concourse.tile as tile
from concourse.bass_utils import run_bass_kernel_spmd

# ---- fixed problem hyperparameters (from the nn.Module) ----
F_IN = 256
H = 8
C = 32
NCLS = 40
NEG = 0.2

W = 8               # cores
P = 128             # partitions
D1 = F_IN + 2 * H   # HAUG row: [h (256) | a_src (8) | a_dst (8)] = 272
D2 = NCLS + 8       # T2 row: [h2+b2 (40) | a2_src (1) | a2_dst x7 (41:48)]
R = 4               # edge tiles batched per compute group

f32 = mybir.dt.float32
bf16 = mybir.dt.bfloat16
i32 = mybir.dt.int32
BF_NP = mybir.dt.np(bf16)

Exp = mybir.ActivationFunctionType.Exp
Copy = mybir.ActivationFunctionType.Copy
ADD = mybir.AluOpType.add
MULT = mybir.AluOpType.mult
MAX = mybir.AluOpType.max

TRACE = False       # set by test harness for profiling runs
_CACHE = {}


def _host_prep(x, edge_index):
    """Index-only preprocessing. Returns (meta, per-core arrays)."""
    N = x.shape[0]
    E = edge_index.shape[1]
    src_a = np.asarray(edge_index[0], np.int64)
    dst_a = np.asarray(edge_index[1], np.int64)
    order = np.argsort(dst_a, kind="stable")
    src_s = src_a[order].astype(np.int64)
    dst_s = dst_a[order].astype(np.int64)

    nt_real = -(-N // P)
    T = -(-nt_real // W)
    SH = T * P
    SH2 = SH // 2
    NPAD = W * SH

    # bucket edges by (core, dst tile, src half) for the split AllGather
    gt = dst_s // P                      # global dst tile
    half = (src_s % SH) // SH2           # 0 / 1
    key = gt * 2 + half
    cnt = np.bincount(key, minlength=W * T * 2).reshape(W, T, 2)
    K01 = -(-cnt.max(axis=0) // P)       # [T, 2] tiles per bucket
    K01[:, 0] = np.maximum(K01[:, 0], 1)
    K0 = K01[:, 0]
    K1 = K01[:, 1]
    K = K0 + K1
    offs = np.zeros(T + 1, np.int64)
    offs[1:] = np.cumsum(K)
    CT = int(offs[-1])

    # rank of each edge within its (core, tile, half) bucket
    sort2 = np.argsort(key, kind="stable")
    src_s = src_s[sort2]
    dst_s = dst_s[sort2]
    gt = gt[sort2]
    half = half[sort2]
    key = key[sort2]
    start_k = np.zeros(W * T * 2 + 1, np.int64)
    start_k[1:] = np.cumsum(cnt.reshape(-1))
    q = np.arange(len(dst_s)) - start_k[key]
    c_all = gt // T
    t_all = gt % T
    col = offs[t_all] + np.where(half == 0, 0, K0[t_all]) + q // P
    p_all = q % P

    src_arr = np.zeros((W, P, CT), np.int64)
    dstl_arr = np.full((W, P, CT), -1, np.int16)
    src_arr[c_all, p_all, col] = src_s
    dstl_arr[c_all, p_all, col] = (dst_s % P).astype(np.int16)

    # layer-2 gather rows in the split-T2F layout:
    # node v = s*SH + l -> T2F[l // SH2] row s*SH2 + (l % SH2)
    t2row = (src_arr // SH) * SH2 + (src_arr % SH) % SH2
    t2row = t2row.astype(np.int32)
    src_arr = src_arr.astype(np.int32)

    # chunk table: one entry per (t, j0) compute group
    chunks = []                          # (t, base, j0, r)
    chunk_of = {}
    for t in range(T):
        for j0 in range(0, int(K[t]), R):
            r = min(R, int(K[t]) - j0)
            chunk_of[(t, j0)] = len(chunks)
            chunks.append((t, int(offs[t]), j0, r))
    NCHT = len(chunks)

    offs0 = np.zeros(T + 1, np.int64)
    offs0[1:] = np.cumsum(K0)
    CT0 = int(offs0[-1])

    meta = dict(N=N, E=E, T=T, SH=SH, SH2=SH2, NPAD=NPAD,
                K=tuple(int(k) for k in K), K0=tuple(int(k) for k in K0),
                offs=offs, offs0=offs0, CT=CT, CT0=CT0, NCHT=NCHT,
                chunks=tuple(chunks), chunk_of=chunk_of)

    xb = np.zeros((NPAD, F_IN), BF_NP)
    xb[:N] = np.asarray(x, np.float32).astype(BF_NP)

    def build_xetc(c):
        # [NCHT, 2, P(feat), R, P(edge)] chunk-blocked transposed x rows
        out = np.zeros((NCHT, 2, P, R, P), BF_NP)
        for ci, (t, base, j0, r) in enumerate(chunks):
            cols = src_arr[c][:, base + j0:base + j0 + r]     # [P(edge), r]
            blk = xb[cols]                                    # [Pe, r, 256]
            out[ci, :, :, :r, :] = (blk.transpose(2, 1, 0)    # [256, r, Pe]
                                    .reshape(2, P, r, P))
        return out

    xts = [np.ascontiguousarray(xb[c * SH:(c + 1) * SH].T) for c in range(W)]

    arrays = dict(src_arr=src_arr, dstl_arr=dstl_arr, t2row=t2row,
                  build_xetc=build_xetc, xts=xts)
    return meta, arrays


def _make_sstc(dstl_core, meta):
    """[P, CT] int16 dst-local (-1 pad) -> [NCHT, P, R, 256] bf16 [S | S^T]."""
    chunks = meta["chunks"]
    NCHT = meta["NCHT"]
    iota = np.arange(P, dtype=np.int16)
    out = np.zeros((NCHT, P, R, 2 * P), BF_NP)
    for ci, (t, base, j0, r) in enumerate(chunks):
        d = dstl_core[:, base + j0:base + j0 + r].T          # [r, 128e]
        S = (d[:, None, :] == iota[None, :, None])           # [r, d, e]
        out[ci, :, :r, :P] = S.transpose(1, 0, 2)
        out[ci, :, :r, P:] = S.transpose(2, 0, 1)
    return out


def _edge_chunks(nc, meta, consts, *, t, selfrow, getg, dglen, nheads, hw,
                 agg):
    """Shared per-chunk compute: attention weights + incidence aggregation."""
    K = meta["K"]
    chunk_of = meta["chunk_of"]
    sstc_d = consts["sstc_d"]
    pool = consts["pool"]
    spool = consts["spool"]
    apsum = consts["apsum"]
    DA = dglen + nheads
    nj = K[t]

    for j0 in range(0, nj, R):
        r = min(R, nj - j0)
        ci = chunk_of[(t, j0)]
        g = getg(j0, r)
        sst = spool.tile([P, R, 2 * P], bf16, tag="sst")
        nc.scalar.dma_start(out=sst[:, :r, :], in_=sstc_d[ci, :, :r, :])
        ade = apsum.tile([P, R, nheads], f32, tag="ade")
        for ri in range(r):
            nc.tensor.matmul(
                ade[:, ri, :], lhsT=sst[:, ri, :P],
                rhs=selfrow[:, dglen + nheads:dglen + 2 * nheads],
                start=True, stop=True)
        s = pool.tile([P, R, nheads], f32, tag="s")
        nc.vector.tensor_add(out=s[:, :r], in0=g[:, :r, dglen:dglen + nheads],
                             in1=ade[:, :r])
        e = pool.tile([P, R, nheads], f32, tag="e")
        nc.vector.scalar_tensor_tensor(
            out=e[:, :r], in0=s[:, :r], scalar=NEG, in1=s[:, :r],
            op0=MULT, op1=MAX)
        rhs = pool.tile([P, R, DA], bf16, tag="rhs")
        nc.scalar.activation(out=rhs[:, :r, dglen:], in_=e[:, :r], func=Exp)
        nc.vector.tensor_tensor(
            out=rhs[:, :r, :dglen].rearrange("p r (h c) -> p r h c", h=nheads),
            in0=g[:, :r, :dglen].rearrange("p r (h c) -> p r h c", h=nheads),
            in1=rhs[:, :r, dglen:].rearrange("p r (h o) -> p r h o", o=1)
                .to_broadcast([P, r, nheads, hw]),
            op=MULT)
        for ri in range(r):
            nc.tensor.matmul(
                agg[:], lhsT=sst[:, ri, P:], rhs=rhs[:, ri, :],
                start=(j0 == 0 and ri == 0), stop=(j0 + ri == nj - 1))


def _build_program(meta):
    T, SH, SH2, NPAD = meta["T"], meta["SH"], meta["SH2"], meta["NPAD"]
    CT, CT0, NCHT = meta["CT"], meta["CT0"], meta["NCHT"]
    K, K0, offs, offs0 = meta["K"], meta["K0"], meta["offs"], meta["offs0"]
    chunk_of = meta["chunk_of"]
    K0MAX = max(max(K0), 1)

    nc = bacc.Bacc("TRN2", target_bir_lowering=False, debug=False, num_devices=W)

    xetc_d = nc.dram_tensor("XETC", [NCHT, 2, P, R, P], bf16, kind="ExternalInput")
    xts_d = nc.dram_tensor("xTs", [F_IN, SH], bf16, kind="ExternalInput")
    w1_d = nc.dram_tensor("W1", [F_IN, F_IN], f32, kind="ExternalInput")
    asrc_d = nc.dram_tensor("asrc", [1, F_IN], f32, kind="ExternalInput")
    adstv_d = nc.dram_tensor("adstv", [1, F_IN], f32, kind="ExternalInput")
    b1_d = nc.dram_tensor("b1", [1, F_IN], f32, kind="ExternalInput")
    w2_d = nc.dram_tensor("W2", [F_IN, NCLS], f32, kind="ExternalInput")
    a2s_d = nc.dram_tensor("a2s", [1, NCLS], f32, kind="ExternalInput")
    a2d_d = nc.dram_tensor("a2d", [1, NCLS], f32, kind="ExternalInput")
    b2_d = nc.dram_tensor("b2", [1, NCLS], f32, kind="ExternalInput")
    srcg2_d = nc.dram_tensor("srcg2", [P, CT], i32, kind="ExternalInput")
    sstc_d = nc.dram_tensor("sstc", [NCHT, P, R, 2 * P], bf16, kind="ExternalInput")
    out_d = nc.dram_tensor("out", [SH, NCLS], f32, kind="ExternalOutput")

    HAUGs = nc.dram_tensor("HAUGs", [SH, D1], bf16)
    Z = nc.dram_tensor("Z", [SH, F_IN], bf16)
    GE0 = nc.dram_tensor("GE0", [P, max(CT0, 1), D2], bf16)

    with tile.TileContext(nc) as tc:
        with contextlib.ExitStack() as top:
            cpool = top.enter_context(tc.tile_pool(name="const", bufs=1))
            dram = top.enter_context(tc.tile_pool(name="dram", bufs=1, space="DRAM"))

            srcb2 = cpool.tile([P, CT], i32)
            nc.sync.dma_start(out=srcb2[:], in_=srcg2_d[:])

            rhs1 = [cpool.tile([P, D1], bf16, name=f"rhs1_{k}") for k in range(2)]
            rhs2 = [cpool.tile([P, D2], bf16, name=f"rhs2_{k}") for k in range(2)]
            b1_b = cpool.tile([P, F_IN], f32)
            b2p_b = cpool.tile([P, D2], f32)

            # ---- setup: broadcast rows + fold attention vectors into rhs ----
            with contextlib.ExitStack() as su:
                spool = su.enter_context(tc.tile_pool(name="setup", bufs=1))
                spsum = su.enter_context(tc.tile_pool(name="setup_ps", bufs=1, space="PSUM"))
                ones = spool.tile([1, P], f32)
                nc.vector.memset(ones[:], 1.0)

                def bcast(dram_ap, width, out_ap):
                    ps = spsum.tile([P, width], f32, tag="bps")
                    row = spool.tile([1, width], f32, tag="brow")
                    nc.sync.dma_start(out=row[:], in_=dram_ap)
                    nc.tensor.matmul(ps[:], lhsT=ones[:], rhs=row[:], start=True, stop=True)
                    nc.vector.tensor_copy(out=out_ap, in_=ps[:])

                asrc_b = spool.tile([P, F_IN], f32)
                bcast(asrc_d[:], F_IN, asrc_b[:])
                adst_b = spool.tile([P, F_IN], f32)
                bcast(adstv_d[:], F_IN, adst_b[:])
                bcast(b1_d[:], F_IN, b1_b[:])
                a2s_b = spool.tile([P, NCLS], f32)
                bcast(a2s_d[:], NCLS, a2s_b[:])
                a2d_b = spool.tile([P, NCLS], f32)
                bcast(a2d_d[:], NCLS, a2d_b[:])
                nc.vector.memset(b2p_b[:], 0.0)
                bcast(b2_d[:], NCLS, b2p_b[:, :NCLS])

                for k in range(2):
                    w1sb = spool.tile([P, F_IN], f32, tag="w1sb")
                    nc.sync.dma_start(out=w1sb[:], in_=w1_d[k * P:(k + 1) * P, :])
                    nc.vector.tensor_copy(out=rhs1[k][:, :F_IN], in_=w1sb[:])
                    for vec_b, col in ((asrc_b, F_IN), (adst_b, F_IN + H)):
                        tmp = spool.tile([P, F_IN], f32, tag="tmp")
                        nc.vector.tensor_mul(out=tmp[:], in0=w1sb[:], in1=vec_b[:])
                        vred = spool.tile([P, H], f32, tag="vred")
                        nc.vector.tensor_reduce(
                            out=vred[:], in_=tmp[:].rearrange("p (h c) -> p h c", h=H),
                            axis=mybir.AxisListType.X, op=ADD)
                        nc.vector.tensor_copy(out=rhs1[k][:, col:col + H], in_=vred[:])

                    w2sb = spool.tile([P, NCLS], f32, tag="w2sb")
                    nc.sync.dma_start(out=w2sb[:], in_=w2_d[k * P:(k + 1) * P, :])
                    nc.vector.tensor_copy(out=rhs2[k][:, :NCLS], in_=w2sb[:])
                    for vec_b, cs in ((a2s_b, slice(NCLS, NCLS + 1)),
                                      (a2d_b, slice(NCLS + 1, D2))):
                        tmp2 = spool.tile([P, NCLS], f32, tag="tmp2")
                        nc.vector.tensor_mul(out=tmp2[:], in0=w2sb[:], in1=vec_b[:])
                        vred2 = spool.tile([P, 1], f32, tag="vred2")
                        nc.vector.tensor_reduce(
                            out=vred2[:], in_=tmp2[:].rearrange("p (o c) -> p o c", o=1),
                            axis=mybir.AxisListType.X, op=ADD)
                        n_rep = cs.stop - cs.start
                        nc.vector.tensor_copy(
                            out=rhs2[k][:, cs], in_=vred2[:].to_broadcast([P, n_rep]))

            # ---- Phase A: HAUGs (bias-free) for own dst shard ----
            with contextlib.ExitStack() as pa:
                apool = pa.enter_context(tc.tile_pool(name="pa", bufs=4))
                apsum = pa.enter_context(tc.tile_pool(name="pa_ps", bufs=2, space="PSUM"))
                BS = min(14, T)
                for b0 in range(0, T, BS):
                    nb = min(BS, T - b0)
                    xt = [apool.tile([P, BS * P], bf16, tag=f"xt{k}", name=f"xt{k}")
                          for k in range(2)]
                    for k in range(2):
                        nc.sync.dma_start(
                            out=xt[k][:, :nb * P],
                            in_=xts_d[k * P:(k + 1) * P, b0 * P:(b0 + nb) * P])
                    hsb = apool.tile([P, BS, D1], bf16, tag="hsb")
                    for nt in range(nb):
                        ps = apsum.tile([P, D1], f32, tag="aps")
                        for k in range(2):
                            nc.tensor.matmul(
                                ps[:], lhsT=xt[k][:, nt * P:(nt + 1) * P], rhs=rhs1[k][:],
                                start=(k == 0), stop=(k == 1))
                        nc.scalar.activation(out=hsb[:, nt, :], in_=ps[:],
                                             func=Copy)
                    row0 = b0 * P
                    nc.scalar.dma_start(
                        out=HAUGs[row0:row0 + nb * P, :].rearrange("(a p) d -> p a d", p=P),
                        in_=hsb[:, :nb, :])

            cpool2 = top.enter_context(tc.tile_pool(name="pc", bufs=3))
            cpsum = top.enter_context(tc.tile_pool(name="pc_ps", bufs=1, space="PSUM"))
            T2L = dram.tile([SH, D2], bf16, name="T2L")
            ep_pool = top.enter_context(tc.tile_pool(name="ep", bufs=8))
            ep_spool = top.enter_context(tc.tile_pool(name="ep_s", bufs=6))
            ep_fpool = top.enter_context(tc.tile_pool(name="ep_f", bufs=3))
            ep_psum = top.enter_context(tc.tile_pool(name="ep_p", bufs=2, space="PSUM"))
            ep_apsum = top.enter_context(tc.tile_pool(name="ep_a", bufs=2, space="PSUM"))
            ep_hpsum = top.enter_context(tc.tile_pool(name="ep_h", bufs=3, space="PSUM"))
            consts = dict(sstc_d=sstc_d, pool=ep_pool, spool=ep_spool,
                          fpool=ep_fpool, psum=ep_psum, apsum=ep_apsum)

            # T2 AllGather halves (layer-2 gather tables)
            T2F0 = dram.tile([W * SH2, D2], bf16, name="T2F0", addr_space="Shared")
            T2F1 = dram.tile([W * SH2, D2], bf16, name="T2F1", addr_space="Shared")

            # ---- Phase B: layer-1 edge phase (dense) -> Z, T2L ----
            def flush1(t, agg, selfrow, fpool):
                es = fpool.tile([P, H], f32, tag="es")
                nc.vector.tensor_add(out=es[:], in0=selfrow[:, F_IN:F_IN + H],
                                     in1=selfrow[:, F_IN + H:])
                nc.vector.scalar_tensor_tensor(
                    out=es[:], in0=es[:], scalar=NEG, in1=es[:], op0=MULT, op1=MAX)
                exs = fpool.tile([P, H], f32, tag="exs")
                nc.scalar.activation(out=exs[:], in_=es[:], func=Exp)
                selfsc = fpool.tile([P, F_IN], f32, tag="selfsc")
                nc.vector.tensor_tensor(
                    out=selfsc[:].rearrange("p (h c) -> p h c", h=H),
                    in0=selfrow[:, :F_IN].rearrange("p (h c) -> p h c", h=H),
                    in1=exs[:].rearrange("p (h o) -> p h o", o=1).to_broadcast([P, H, C]),
                    op=MULT)
                numer = fpool.tile([P, F_IN], f32, tag="numer")
                nc.vector.tensor_add(out=numer[:], in0=selfsc[:], in1=agg[:, :F_IN])
                dinv = fpool.tile([P, H], f32, tag="dinv")
                nc.vector.tensor_add(out=dinv[:], in0=exs[:], in1=agg[:, F_IN:])
                nc.vector.tensor_scalar_add(out=dinv[:], in0=dinv[:], scalar1=1e-16)
                nc.vector.reciprocal(out=dinv[:], in_=dinv[:])
                o = fpool.tile([P, F_IN], f32, tag="o")
                nc.vector.tensor_tensor(
                    out=o[:].rearrange("p (h c) -> p h c", h=H),
                    in0=numer[:].rearrange("p (h c) -> p h c", h=H),
                    in1=dinv[:].rearrange("p (h o) -> p h o", o=1).to_broadcast([P, H, C]),
                    op=MULT)
                nc.vector.tensor_add(out=o[:], in0=o[:], in1=b1_b[:])
                mmin = fpool.tile([P, F_IN], f32, tag="mmin")
                nc.vector.tensor_scalar_min(out=mmin[:], in0=o[:], scalar1=0.0)
                ex = fpool.tile([P, F_IN], f32, tag="ex")
                nc.scalar.activation(out=ex[:], in_=mmin[:], func=Exp)
                rel = fpool.tile([P, F_IN], f32, tag="rel")
                nc.vector.tensor_scalar_max(out=rel[:], in0=o[:], scalar1=0.0)
                z = fpool.tile([P, F_IN], bf16, tag="z")
                nc.vector.scalar_tensor_tensor(
                    out=z[:], in0=ex[:], scalar=-1.0, in1=rel[:], op0=ADD, op1=ADD)
                nc.sync.dma_start(out=Z[t * P:(t + 1) * P, :], in_=z[:])
                # Phase C for this tile, overlapped with the edge phase
                zt = [cpool2.tile([P, P], bf16, tag=f"zt{k}", name=f"zt{k}")
                      for k in range(2)]
                for k in range(2):
                    nc.sync.dma_start(
                        out=zt[k][:], in_=Z[t * P:(t + 1) * P, k * P:(k + 1) * P],
                        transpose=True)
                ps2 = cpsum.tile([P, D2], f32, tag="cps")
                for k in range(2):
                    nc.tensor.matmul(ps2[:], lhsT=zt[k][:], rhs=rhs2[k][:],
                                     start=(k == 0), stop=(k == 1))
                t2sb = cpool2.tile([P, D2], bf16, tag="t2sb")
                nc.vector.tensor_add(out=t2sb[:], in0=ps2[:], in1=b2p_b[:])
                nc.sync.dma_start(out=T2L[t * P:(t + 1) * P, :], in_=t2sb[:])

            t2f0_ap = T2F0.tensor.ap()
            t2f1_ap = T2F1.tensor.ap()

            def stage0(t, stpool):
                """Gather this tile's bucket-0 T2 rows and stage them to DRAM
                (runs on gpsimd+sync, concurrent with phase-B compute)."""
                k0 = K0[t]
                if k0 == 0:
                    return
                b0 = int(offs0[t])
                base = int(offs[t])
                ring = stpool.tile([P, K0MAX, D2], bf16, tag="st")
                for j in range(k0):
                    c1 = base + j
                    nc.gpsimd.indirect_dma_start(
                        out=ring[:, j, :], out_offset=None, in_=t2f0_ap[:],
                        in_offset=bass.IndirectOffsetOnAxis(
                            ap=srcb2[:, c1:c1 + 1], axis=0),
                    )
                nc.sync.dma_start(out=GE0[:, b0:b0 + k0, :], in_=ring[:, :k0, :])

            def phase_b_tile(t, bpool):
                agg = ep_psum.tile([P, F_IN + H], f32, tag="agg")
                selfrow = ep_fpool.tile([P, D1], bf16, tag="selfrow")
                nc.sync.dma_start(out=selfrow[:],
                                  in_=HAUGs[t * P:(t + 1) * P, :])

                def getg1(j0, r, t=t):
                    ci = chunk_of[(t, j0)]
                    hb = bpool.tile([P, R, D1], bf16, tag="hb")
                    xe = bpool.tile([P, 2, R, P], bf16, tag="xe")
                    for k in range(2):
                        nc.sync.dma_start(out=xe[:, k, :r, :],
                                          in_=xetc_d[ci, k, :, :r, :])
                    for ri in range(r):
                        hp = ep_hpsum.tile([P, D1], f32, tag="hp")
                        for k in range(2):
                            nc.tensor.matmul(
                                hp[:], lhsT=xe[:, k, ri, :], rhs=rhs1[k][:],
                                start=(k == 0), stop=(k == 1))
                        if ri % 2 == 0:
                            nc.scalar.activation(out=hb[:, ri, :], in_=hp[:],
                                                 func=Copy)
                        else:
                            nc.vector.tensor_copy(out=hb[:, ri, :], in_=hp[:])
                    return hb[:]

                _edge_chunks(nc, meta, consts, t=t, selfrow=selfrow,
                             getg=getg1, dglen=F_IN, nheads=H, hw=C, agg=agg)
                flush1(t, agg, selfrow, ep_fpool)

            TH = (T + 1) // 2
            with contextlib.ExitStack() as pb:
                bpool = pb.enter_context(tc.tile_pool(name="pb", bufs=8))
                stpool = pb.enter_context(tc.tile_pool(name="pst", bufs=4))
                for t in range(TH):
                    phase_b_tile(t, bpool)
                # first-half T2 rows are final -> AllGather half 0
                nc.gpsimd.collective_compute(
                    "AllGather", mybir.AluOpType.bypass,
                    replica_groups=[list(range(W))],
                    ins=[T2L[0:SH2, :]], outs=[T2F0.opt()])
                # second half of phase B, with bucket-0 layer-2 gathers
                # (2 staging tiles per compute tile) interleaved
                st_t = 0
                for t in range(TH, T):
                    phase_b_tile(t, bpool)
                    for _ in range(2):
                        if st_t < T:
                            stage0(st_t, stpool)
                            st_t += 1
                while st_t < T:
                    stage0(st_t, stpool)
                    st_t += 1
                nc.gpsimd.collective_compute(
                    "AllGather", mybir.AluOpType.bypass,
                    replica_groups=[list(range(W))],
                    ins=[T2L[SH2:SH, :]], outs=[T2F1.opt()])

            # ---- Phase D: layer-2 edge phase -> out ----
            def flush2(t, agg, selfrow, fpool):
                es = fpool.tile([P, 1], f32, tag="es2")
                nc.vector.tensor_add(out=es[:], in0=selfrow[:, NCLS:NCLS + 1],
                                     in1=selfrow[:, NCLS + 1:NCLS + 2])
                nc.vector.scalar_tensor_tensor(
                    out=es[:], in0=es[:], scalar=NEG, in1=es[:], op0=MULT, op1=MAX)
                exs = fpool.tile([P, 1], f32, tag="exs2")
                nc.scalar.activation(out=exs[:], in_=es[:], func=Exp)
                selfsc = fpool.tile([P, NCLS], f32, tag="selfsc2")
                nc.vector.tensor_tensor(
                    out=selfsc[:], in0=selfrow[:, :NCLS],
                    in1=exs[:].to_broadcast([P, NCLS]), op=MULT)
                numer = fpool.tile([P, NCLS], f32, tag="numer2")
                nc.vector.tensor_add(out=numer[:], in0=selfsc[:], in1=agg[:, :NCLS])
                dinv = fpool.tile([P, 1], f32, tag="dinv2")
                nc.vector.tensor_add(out=dinv[:], in0=exs[:], in1=agg[:, NCLS:])
                nc.vector.tensor_scalar_add(out=dinv[:], in0=dinv[:], scalar1=1e-16)
                nc.vector.reciprocal(out=dinv[:], in_=dinv[:])
                o = fpool.tile([P, NCLS], f32, tag="o2")
                nc.vector.tensor_tensor(
                    out=o[:], in0=numer[:], in1=dinv[:].to_broadcast([P, NCLS]), op=MULT)
                nc.sync.dma_start(out=out_d[t * P:(t + 1) * P, :], in_=o[:])

            with contextlib.ExitStack() as pd:
                dpool = pd.enter_context(tc.tile_pool(name="pd", bufs=8))
                for t in range(T):
                    agg = ep_psum.tile([P, NCLS + 1], f32, tag="agg")
                    base = int(offs[t])
                    b0 = int(offs0[t])
                    k0 = K0[t]
                    selfrow = ep_fpool.tile([P, D2], bf16, tag="selfrow2")
                    nc.sync.dma_start(out=selfrow[:],
                                      in_=T2L[t * P:(t + 1) * P, :])

                    def getg2(j0, r, base=base, b0=b0, k0=k0):
                        g = dpool.tile([P, R, D2], bf16, tag="g2")
                        nb0 = max(0, min(k0 - j0, r))
                        if nb0 > 0:         # staged bucket-0 rows
                            nc.sync.dma_start(
                                out=g[:, :nb0, :],
                                in_=GE0[:, b0 + j0:b0 + j0 + nb0, :])
                        for ri in range(nb0, r):   # bucket-1 gathers
                            c1 = base + j0 + ri
                            nc.gpsimd.indirect_dma_start(
                                out=g[:, ri, :], out_offset=None, in_=t2f1_ap[:],
                                in_offset=bass.IndirectOffsetOnAxis(
                                    ap=srcb2[:, c1:c1 + 1], axis=0),
                            )
                        return g[:]

                    _edge_chunks(nc, meta, consts, t=t, selfrow=selfrow,
                                 getg=getg2, dglen=NCLS, nheads=1, hw=NCLS,
                                 agg=agg)
                    flush2(t, agg, selfrow, ep_fpool)

    nc.compile()
    return nc


def kernel(**inputs):
    x = np.asarray(inputs["x"], np.float32)
    edge_index = np.asarray(inputs["edge_index"])
    meta, arrays = _host_prep(x, edge_index)

    key = (meta["N"], meta["E"], meta["K"], meta["K0"])
    if key not in _CACHE:
        _CACHE[key] = _build_program(meta)
    nc = _CACHE[key]

    common = {
        "W1": np.asarray(inputs["W1"], np.float32),
        "asrc": np.asarray(inputs["att_src1"], np.float32).reshape(1, -1),
        "adstv": np.asarray(inputs["att_dst1"], np.float32).reshape(1, -1),
        "b1": np.asarray(inputs["bias1"], np.float32).reshape(1, -1),
        "W2": np.asarray(inputs["W2"], np.float32),
        "a2s": np.asarray(inputs["att_src2"], np.float32).reshape(1, -1),
        "a2d": np.asarray(inputs["att_dst2"], np.float32).reshape(1, -1),
        "b2": np.asarray(inputs["bias2"], np.float32).reshape(1, -1),
    }
    in_maps = []
    for c in range(W):
        m = dict(common)
        m["XETC"] = arrays["build_xetc"](c)
        m["xTs"] = arrays["xts"][c]
        m["srcg2"] = arrays["t2row"][c]
        m["sstc"] = _make_sstc(arrays["dstl_arr"][c], meta)
        in_maps.append(m)

    res = run_bass_kernel_spmd(nc, in_maps, core_ids=list(range(W)), trace=TRACE)
    kernel.last_results = res

    N = meta["N"]
    out = np.concatenate([res.results[c]["out"] for c in range(W)], axis=0)
    return np.ascontiguousarray(out[:N])
